# revision 2
# baseline (speedup 1.0000x reference)
"""Trainium2 Bass kernel for nn_Attention_12403865551261.

Causal multi-head attention (B=1, S=4096, E=768, H=12, Dh=64, no 1/sqrt(dh)
scaling) sharded over 8 NeuronCores.

Sharding: queries row-sharded (512 contiguous rows per core). Each core
computes Q/K/V for its own rows (Megatron-ish: projections in transposed
layout so no on-device transposes are needed anywhere), K^T and V are
AllGathered across the 8 cores in bf16, then each core runs
softmax(Q K^T + mask_bias) V for its own rows and the output projection.

Layout trick: scores are computed as S_T[k, q] = (K^T)^T-free matmul with
dh on the contraction dim, so softmax normalization sums land on the
partition axis and are computed for free by appending a ones-column to V
(row 64 of the attention output accumulator is then the softmax
denominator). The attention output is produced directly in transposed
[e, q] layout, which is exactly the lhsT the output projection needs.

Mask handling: host converts the bool mask into additive bias tiles
(0 / -30000) fed per-core as input data; exp(s - 30000) == 0.0 in f32.
"""
import sys
import os
import threading

sys.path.insert(0, "/opt/trn_rl_repo")

import numpy as np
import ml_dtypes

import concourse.bass as bass
import concourse.bacc as bacc
import concourse.tile as tile
import concourse.mybir as mybir
from concourse.bass_utils import run_bass_kernel_spmd

BF16 = mybir.dt.bfloat16
F32 = mybir.dt.float32
NP_BF16 = ml_dtypes.bfloat16

N_CORES = 8
S = 4096
E = 768
H = 12
DH = 64
E3 = 3 * E          # 2304
SE = 896            # contraction dim padded: 768 weights + 1 bias row + pad -> 7*128
KC = SE // 128      # 7 contraction chunks
ROWS = S // N_CORES  # 512 own rows per core
NEG = -30000.0

VW = 65             # V columns per head incl. ones-column
G_W = 1024          # free width of one score-psum group (4 k-chunks x 256 q)


def build_program():
    nc = bacc.Bacc("TRN2", target_bir_lowering=False, debug=False,
                   num_devices=N_CORES)

    # ---- I/O ----
    xT = nc.dram_tensor("xT", [SE, ROWS], BF16, kind="ExternalInput")
    wqkvT = nc.dram_tensor("wqkvT", [SE, E3], BF16, kind="ExternalInput")
    woutT = nc.dram_tensor("woutT", [E, E], BF16, kind="ExternalInput")
    sbias = nc.dram_tensor("sbias", [2, 8, 128, G_W], BF16, kind="ExternalInput")
    out_own = nc.dram_tensor("out_own", [ROWS, E], F32, kind="ExternalOutput")

    with tile.TileContext(nc) as tc:
        with (
            tc.tile_pool(name="const", bufs=1) as cpool,      # long-lived SBUF
            tc.tile_pool(name="dram", bufs=1, space="DRAM") as dram,
        ):
            # ---------------- phase 1: QKV projection (own rows) -----------
            qt_sb = [cpool.tile([128, ROWS], BF16, name=f"qt{t}") for t in range(6)]
            attnT_sb = [cpool.tile([128, ROWS], BF16, name=f"at{t}") for t in range(6)]
            woutT_sb = [cpool.tile([128, E], BF16, name=f"wo{t}") for t in range(6)]
            ones_sb = cpool.tile([1, 64], F32, name="ones")
            nc.vector.memset(ones_sb[:], 1.0)
            for t in range(6):
                nc.sync.dma_start(woutT_sb[t][:], woutT[128 * t:128 * (t + 1), :])

            kt_contrib = dram.tile([E, ROWS], BF16)
            v_contrib = dram.tile([ROWS, H * VW], BF16)
            kt_g = dram.tile([N_CORES * E, ROWS], BF16, addr_space="Shared")
            v_g = dram.tile([S, H * VW], BF16, addr_space="Shared")

            with (
                tc.tile_pool(name="proj", bufs=1) as ppool,
                tc.tile_pool(name="pq", bufs=2, space="PSUM") as pq,
                tc.tile_pool(name="pv", bufs=2, space="PSUM") as pv,
            ):
                w_sb = [ppool.tile([128, E3], BF16, name=f"w{t}") for t in range(KC)]
                x_sb = [ppool.tile([128, ROWS], BF16, name=f"x{t}") for t in range(KC)]
                for t in range(KC):
                    nc.sync.dma_start(w_sb[t][:], wqkvT[128 * t:128 * (t + 1), :])
                    nc.sync.dma_start(x_sb[t][:], xT[128 * t:128 * (t + 1), :])

                kt_own = [ppool.tile([128, ROWS], BF16, name=f"kt{t}") for t in range(6)]
                v_own = [ppool.tile([128, H * VW], BF16, name=f"v{t}") for t in range(4)]

                # Q^T and K^T chunks: out part = qkv dim, free = own rows
                for oc in range(12):
                    ps = pq.tile([128, ROWS], F32, tag="pq")
                    for kc in range(KC):
                        nc.tensor.matmul(
                            ps[:],
                            w_sb[kc][:, 128 * oc:128 * (oc + 1)],
                            x_sb[kc][:],
                            start=(kc == 0), stop=(kc == KC - 1),
                        )
                    dst = qt_sb[oc] if oc < 6 else kt_own[oc - 6]
                    nc.vector.tensor_copy(dst[:], ps[:])

                # V chunks: out part = own rows, free = v dims (all heads)
                for sc in range(4):
                    ps = pv.tile([128, E], F32, tag="pv")
                    for kc in range(KC):
                        nc.tensor.matmul(
                            ps[:, 0:512],
                            x_sb[kc][:, 128 * sc:128 * (sc + 1)],
                            w_sb[kc][:, 2 * E:2 * E + 512],
                            start=(kc == 0), stop=(kc == KC - 1),
                        )
                        nc.tensor.matmul(
                            ps[:, 512:768],
                            x_sb[kc][:, 128 * sc:128 * (sc + 1)],
                            w_sb[kc][:, 2 * E + 512:3 * E],
                            start=(kc == 0), stop=(kc == KC - 1),
                        )
                    nc.vector.memset(v_own[sc][:], 1.0)
                    for h in range(H):
                        nc.vector.tensor_copy(
                            v_own[sc][:, VW * h:VW * h + DH],
                            ps[:, DH * h:DH * (h + 1)],
                        )

                # contributions to DRAM for the all-gather
                for t in range(6):
                    nc.sync.dma_start(kt_contrib[128 * t:128 * (t + 1), :], kt_own[t][:])
                for sc in range(4):
                    nc.sync.dma_start(v_contrib[128 * sc:128 * (sc + 1), :], v_own[sc][:])

                nc.gpsimd.collective_compute(
                    "AllGather", mybir.AluOpType.bypass,
                    replica_groups=[list(range(N_CORES))],
                    ins=[kt_contrib[:]], outs=[kt_g[:]],
                )
                nc.gpsimd.collective_compute(
                    "AllGather", mybir.AluOpType.bypass,
                    replica_groups=[list(range(N_CORES))],
                    ins=[v_contrib[:]], outs=[v_g[:]],
                )

            # ---------------- phase 2: attention -----------------------
            with (
                tc.tile_pool(name="attn", bufs=1) as apool,
                tc.tile_pool(name="ppool2", bufs=3) as p2,
                tc.tile_pool(name="ps_s", bufs=2, space="PSUM") as ps_s,
                tc.tile_pool(name="ps_o", bufs=2, space="PSUM") as ps_o,
                tc.tile_pool(name="ps_po", bufs=1, space="PSUM") as ps_po,
            ):
                # gathered K^T: 6 tiles [128, S]; head h lives in tile h//2,
                # partitions (h%2)*64, col = global key position
                ktf = [apool.tile([128, S], BF16, name=f"ktf{t}") for t in range(6)]
                for t in range(6):
                    for c in range(N_CORES):
                        nc.sync.dma_start(
                            ktf[t][:, ROWS * c:ROWS * (c + 1)],
                            kt_g[E * c + 128 * t:E * c + 128 * (t + 1), :],
                        )
                # gathered V (+ones col): 32 tiles [128, 780], part = key pos
                vtf = [apool.tile([128, H * VW], BF16, name=f"vtf{j}") for j in range(32)]
                for j in range(32):
                    nc.sync.dma_start(vtf[j][:], v_g[128 * j:128 * (j + 1), :])
                # mask bias tiles, resident
                sb_sb = [apool.tile([128, G_W], BF16, name=f"sb{i}") for i in range(16)]
                for qb in range(2):
                    for g in range(8):
                        nc.sync.dma_start(sb_sb[qb * 8 + g][:], sbias[qb, g, :, :])

                rc_sb = [apool.tile([1, 256], F32, name=f"rc{i}") for i in range(4)]

                for h in range(H):
                    t, po = h // 2, (h % 2) * 64
                    for qb in range(2):
                        o_ps = ps_o.tile([128, 256], F32, tag="o")
                        for g in range(8):
                            s_ps = ps_s.tile([128, G_W], F32, tag="s")
                            for c in range(4):
                                kck = 4 * g + c
                                nc.tensor.matmul(
                                    s_ps[:, 256 * c:256 * (c + 1)],
                                    ktf[t][po:po + 64, 128 * kck:128 * (kck + 1)],
                                    qt_sb[t][po:po + 64, 256 * qb:256 * (qb + 1)],
                                    start=True, stop=True,
                                )
                            nc.vector.tensor_add(s_ps[:], s_ps[:], sb_sb[qb * 8 + g][:])
                            p_sb = p2.tile([128, G_W], BF16, tag="p")
                            nc.scalar.activation(p_sb[:], s_ps[:],
                                                 mybir.ActivationFunctionType.Exp)
                            for c in range(4):
                                kck = 4 * g + c
                                nc.tensor.matmul(
                                    o_ps[0:VW, :],
                                    vtf[kck][:, VW * h:VW * (h + 1)],
                                    p_sb[:, 256 * c:256 * (c + 1)],
                                    start=(g == 0 and c == 0),
                                    stop=(g == 7 and c == 3),
                                )
                        # normalize: row 64 of o_ps is the denominator
                        rc = rc_sb[(h * 2 + qb) % 4]
                        nc.vector.reciprocal(rc[:], o_ps[64:65, :])
                        nc.tensor.matmul(o_ps[64:128, :], ones_sb[:], rc[:],
                                         start=True, stop=True)
                        bc = p2.tile([64, 256], F32, tag="bc")
                        nc.scalar.copy(bc[:], o_ps[64:128, :])
                        nc.vector.tensor_mul(
                            attnT_sb[t][po:po + 64, 256 * qb:256 * (qb + 1)],
                            o_ps[0:64, :], bc[:],
                        )

                # ---------------- phase 3: output projection ---------------
                for qc in range(4):
                    ps = ps_po.tile([128, E], F32, tag="po")
                    for ec in range(6):
                        nc.tensor.matmul(
                            ps[:, 0:512],
                            attnT_sb[ec][:, 128 * qc:128 * (qc + 1)],
                            woutT_sb[ec][:, 0:512],
                            start=(ec == 0), stop=(ec == 5),
                        )
                        nc.tensor.matmul(
                            ps[:, 512:768],
                            attnT_sb[ec][:, 128 * qc:128 * (qc + 1)],
                            woutT_sb[ec][:, 512:768],
                            start=(ec == 0), stop=(ec == 5),
                        )
                    o_sb = p2.tile([128, E], F32, tag="osb")
                    nc.vector.tensor_copy(o_sb[:], ps[:])
                    nc.sync.dma_start(out_own[128 * qc:128 * (qc + 1), :], o_sb[:])

    nc.compile()
    return nc


_cache_lock = threading.Lock()
_cached_nc = None


def _get_program():
    global _cached_nc
    with _cache_lock:
        if _cached_nc is None:
            _cached_nc = build_program()
    return _cached_nc


def kernel(x, mask, w_qkv, b_qkv, w_out, b_out, _trace=False):
    x = np.asarray(x)
    mask = np.asarray(mask)
    w_qkv = np.asarray(w_qkv, dtype=np.float32)
    b_qkv = np.asarray(b_qkv, dtype=np.float32)
    w_out = np.asarray(w_out, dtype=np.float32)
    b_out = np.asarray(b_out, dtype=np.float32)

    x2 = x.reshape(S, E).astype(np.float32)

    # host-side prep (cheap, layout only)
    wq = np.zeros((SE, E3), dtype=np.float32)
    wq[:E] = w_qkv.T
    wq[E] = b_qkv
    wq_bf = wq.astype(NP_BF16)

    woutT_np = np.ascontiguousarray(w_out.T).astype(NP_BF16)

    xT_full = np.zeros((SE, S), dtype=np.float32)
    xT_full[:E] = x2.T
    xT_full[E] = 1.0
    xT_bf = xT_full.astype(NP_BF16)

    maskf = mask.astype(bool)
    in_maps = []
    for i in range(N_CORES):
        rows = slice(ROWS * i, ROWS * (i + 1))
        sub = maskf[rows, :]                               # [512 q, 4096 k]
        bias = np.where(sub, np.float32(0.0), np.float32(NEG))
        # [qb, qq, g, c, kk] -> [qb, g, kk, c, qq]
        b5 = bias.reshape(2, 256, 8, 4, 128).transpose(0, 2, 4, 3, 1)
        sb = np.ascontiguousarray(b5).reshape(2, 8, 128, G_W).astype(NP_BF16)
        in_maps.append({
            "xT": np.ascontiguousarray(xT_bf[:, rows]),
            "wqkvT": wq_bf,
            "woutT": woutT_np,
            "sbias": sb,
        })

    nc = _get_program()
    res = run_bass_kernel_spmd(nc, in_maps, core_ids=list(range(N_CORES)),
                               trace=_trace)

    out = np.concatenate([res.results[i]["out_own"] for i in range(N_CORES)],
                         axis=0)
    out = out + b_out[None, :]
    result = out.reshape(1, S, E).astype(np.float32)
    if _trace:
        return result, res
    return result


# revision 5
# speedup vs baseline: 1.3558x; 1.3558x over previous
"""Trainium2 Bass kernel for nn_Attention_12403865551261.

Causal multi-head attention (B=1, S=4096, E=768, H=12, Dh=64, no 1/sqrt(dh)
scaling) sharded over 8 NeuronCores.

Sharding: queries are row-sharded in a causal-balanced zigzag — core i owns
the two 256-row stripes {i, 15-i} of the 16-stripe sequence, so every core
does the same total attention work. Each core projects Q/K/V for its own 512
rows (all projections produced directly in transposed layout so no on-device
transposes are needed), K^T and V are AllGathered across the 8 cores in
bf16, then each core runs causal softmax(Q K^T) V for its own rows (only
the lower-triangle key blocks) and the output projection.

Per-core causal structure (different kv extents per stripe) is expressed as
8 compute-only branch bodies on tc.If(partition_id == i); all DMAs and the
projection/collective/output phases are uniform across cores.

Layout trick: scores are computed transposed, S_T[k, q], with dh on the
contraction dim; softmax denominators come free from a ones-column appended
to V (row 64 of the attention-output accumulator). Attention output is
produced directly as attnT[e, q], which is exactly the lhsT the output
projection wants. Score matmuls contract over only 64 partitions, so the
two heads of a pair are issued at partition offsets 0/64 — the PE runs them
concurrently in different row-groups (and the full-width activity keeps the
HAM clock-gate warm).

Mask handling: the causal structure is static; only each stripe's last
(diagonal) key group gets an additive bias tile (0 / -30000, host data).
exp(s - 30000) == 0.0 exactly in f32. If the mask input is not causal, a
host fallback computes the reference directly.
"""
import sys
import threading

sys.path.insert(0, "/opt/trn_rl_repo")

import numpy as np
import ml_dtypes

import concourse.bacc as bacc
import concourse.tile as tile
import concourse.mybir as mybir
from concourse.bass_utils import run_bass_kernel_spmd

BF16 = mybir.dt.bfloat16
F32 = mybir.dt.float32
NP_BF16 = ml_dtypes.bfloat16

N_CORES = 8
S = 4096
E = 768
H = 12
DH = 64
E3 = 3 * E          # 2304
SE = 896            # contraction dim: 768 weights + 1 bias row + pad -> 7*128
KC = SE // 128      # 7 contraction chunks
ROWS = S // N_CORES  # 512 own rows per core
STR = 256           # stripe height
NEG = -30000.0
VW = 65             # V columns per head incl. ones-column

# core i owns stripes (i, 15-i); stripe s covers global rows [256s, 256s+256)
STRIPES = [(i, 15 - i) for i in range(N_CORES)]


def stripe_owner(j):
    """global stripe j -> (core, slot)"""
    c = min(j, 15 - j)
    return c, (0 if j < 8 else 1)


def build_program():
    nc = bacc.Bacc("TRN2", target_bir_lowering=False, debug=False,
                   num_devices=N_CORES)

    xT = nc.dram_tensor("xT", [SE, ROWS], BF16, kind="ExternalInput")
    wqkvT = nc.dram_tensor("wqkvT", [SE, E3], BF16, kind="ExternalInput")
    woutT = nc.dram_tensor("woutT", [E, E], BF16, kind="ExternalInput")
    dbias = nc.dram_tensor("dbias", [2, 128, 1024], BF16, kind="ExternalInput")
    out_own = nc.dram_tensor("out_own", [ROWS, E], F32, kind="ExternalOutput")

    with tile.TileContext(nc) as tc:
        with (
            tc.tile_pool(name="const", bufs=1) as cpool,
            tc.tile_pool(name="dram", bufs=1, space="DRAM") as dram,
        ):
            qt_sb = [cpool.tile([128, ROWS], BF16, name=f"qt{t}") for t in range(6)]
            attnT_sb = [cpool.tile([128, ROWS], BF16, name=f"at{t}") for t in range(6)]
            woutT_sb = [cpool.tile([128, E], BF16, name=f"wo{t}") for t in range(6)]
            ones_sb = cpool.tile([1, 64], F32, name="ones")
            nc.vector.memset(ones_sb[:], 1.0)
            for t in range(6):
                nc.sync.dma_start(woutT_sb[t][:], woutT[128 * t:128 * (t + 1), :])
            db_sb = [cpool.tile([128, 1024], BF16, name=f"db{sl}") for sl in range(2)]
            for sl in range(2):
                nc.sync.dma_start(db_sb[sl][:], dbias[sl, :, :])

            kt_contrib = dram.tile([E, ROWS], BF16)
            v_contrib = dram.tile([ROWS, H * VW], BF16)
            kt_g = dram.tile([N_CORES * E, ROWS], BF16, addr_space="Shared")
            v_g = dram.tile([S, H * VW], BF16, addr_space="Shared")

            # ---------------- phase 1: QKV projection (own rows) -----------
            with (
                tc.tile_pool(name="proj", bufs=1) as ppool,
                tc.tile_pool(name="pq", bufs=2, space="PSUM") as pq,
                tc.tile_pool(name="pv", bufs=2, space="PSUM") as pv,
            ):
                w_sb = [ppool.tile([128, E3], BF16, name=f"w{t}") for t in range(KC)]
                x_sb = [ppool.tile([128, ROWS], BF16, name=f"x{t}") for t in range(KC)]
                for t in range(KC):
                    nc.sync.dma_start(w_sb[t][:], wqkvT[128 * t:128 * (t + 1), :])
                    nc.sync.dma_start(x_sb[t][:], xT[128 * t:128 * (t + 1), :])

                kt_own = [ppool.tile([128, ROWS], BF16, name=f"kt{t}") for t in range(6)]
                v_own = [ppool.tile([128, H * VW], BF16, name=f"v{t}") for t in range(4)]

                for oc in range(12):
                    ps = pq.tile([128, ROWS], F32, tag="pq")
                    for kc in range(KC):
                        nc.tensor.matmul(
                            ps[:],
                            w_sb[kc][:, 128 * oc:128 * (oc + 1)],
                            x_sb[kc][:],
                            start=(kc == 0), stop=(kc == KC - 1),
                        )
                    dst = qt_sb[oc] if oc < 6 else kt_own[oc - 6]
                    nc.vector.tensor_copy(dst[:], ps[:])

                for sc in range(4):
                    ps = pv.tile([128, E], F32, tag="pv")
                    for kc in range(KC):
                        nc.tensor.matmul(
                            ps[:, 0:512],
                            x_sb[kc][:, 128 * sc:128 * (sc + 1)],
                            w_sb[kc][:, 2 * E:2 * E + 512],
                            start=(kc == 0), stop=(kc == KC - 1),
                        )
                        nc.tensor.matmul(
                            ps[:, 512:768],
                            x_sb[kc][:, 128 * sc:128 * (sc + 1)],
                            w_sb[kc][:, 2 * E + 512:3 * E],
                            start=(kc == 0), stop=(kc == KC - 1),
                        )
                    nc.vector.memset(v_own[sc][:], 1.0)
                    for h in range(H):
                        nc.vector.tensor_copy(
                            v_own[sc][:, VW * h:VW * h + DH],
                            ps[:, DH * h:DH * (h + 1)],
                        )

                for t in range(6):
                    nc.sync.dma_start(kt_contrib[128 * t:128 * (t + 1), :], kt_own[t][:])
                for sc in range(4):
                    nc.sync.dma_start(v_contrib[128 * sc:128 * (sc + 1), :], v_own[sc][:])

                nc.gpsimd.collective_compute(
                    "AllGather", mybir.AluOpType.bypass,
                    replica_groups=[list(range(N_CORES))],
                    ins=[kt_contrib[:]], outs=[kt_g[:]],
                )
                nc.gpsimd.collective_compute(
                    "AllGather", mybir.AluOpType.bypass,
                    replica_groups=[list(range(N_CORES))],
                    ins=[v_contrib[:]], outs=[v_g[:]],
                )

            # ---------------- phase 2: attention ---------------------------
            with (
                tc.tile_pool(name="attn", bufs=1) as apool,
                tc.tile_pool(name="ps_s", bufs=1, space="PSUM") as ps_s,
                tc.tile_pool(name="ps_o", bufs=1, space="PSUM") as ps_o,
            ):
                # gathered K^T -> global-k layout: tile t holds heads 2t,2t+1
                ktf = [apool.tile([128, S], BF16, name=f"ktf{t}") for t in range(6)]
                for t in range(6):
                    for j in range(16):          # global stripe j
                        c, sl = stripe_owner(j)
                        nc.sync.dma_start(
                            ktf[t][:, STR * j:STR * (j + 1)],
                            kt_g[E * c + 128 * t:E * c + 128 * (t + 1),
                                 STR * sl:STR * (sl + 1)],
                        )
                # gathered V -> global-k layout: tile kk = global key chunk
                vtf = [apool.tile([128, H * VW], BF16, name=f"vtf{kk}") for kk in range(32)]
                for kk in range(32):
                    j = kk // 2
                    c, sl = stripe_owner(j)
                    src_row = ROWS * c + STR * sl + 128 * (kk % 2)
                    nc.sync.dma_start(vtf[kk][:], v_g[src_row:src_row + 128, :])

                # shared compute rings (only one branch executes per core)
                s_ring = [ps_s.tile([128, 1024], F32, name=f"sr{j}", tag=f"sr{j}")
                          for j in range(3)]
                o_ring = [ps_o.tile([128, 512], F32, name=f"or{j}", tag=f"or{j}")
                          for j in range(2)]
                p_ring = [apool.tile([128, 1024], BF16, name=f"pr{j}") for j in range(3)]
                dn_ring = [apool.tile([1, 512], F32, name=f"dn{j}") for j in range(2)]
                rc_ring = [apool.tile([64, 512], F32, name=f"rcr{j}") for j in range(2)]

                pid = nc.partition_id()

                def attention_body(core):
                    si = 0  # s/p ring counter
                    oi = 0  # o ring counter
                    for t in range(6):              # head pair (2t, 2t+1)
                        for sl, s in enumerate(STRIPES[core]):
                            n_ch = 2 * (s + 1)      # 128-wide key chunks
                            n_g = (n_ch + 3) // 4
                            o_ps = o_ring[oi % 2]; oi += 1
                            for g in range(n_g):
                                chunks = list(range(4 * g, min(4 * g + 4, n_ch)))
                                W = 256 * len(chunks)
                                s_e = s_ring[si % 3]; si += 1
                                s_o = s_ring[si % 3]; si += 1
                                for ci, kk in enumerate(chunks):
                                    for par, s_ps in ((0, s_e), (1, s_o)):
                                        nc.tensor.matmul(
                                            s_ps[:, 256 * ci:256 * (ci + 1)],
                                            ktf[t][64 * par:64 * par + 64,
                                                   128 * kk:128 * (kk + 1)],
                                            qt_sb[t][64 * par:64 * par + 64,
                                                     STR * sl:STR * (sl + 1)],
                                            start=True, stop=True,
                                        )
                                p_e = p_ring[(si - 2) % 3]
                                p_o = p_ring[(si - 1) % 3]
                                if g == n_g - 1:   # diagonal group: mask bias
                                    nc.vector.tensor_add(
                                        s_e[:, 0:W], s_e[:, 0:W], db_sb[sl][:, 0:W])
                                    nc.vector.tensor_add(
                                        s_o[:, 0:W], s_o[:, 0:W], db_sb[sl][:, 0:W])
                                nc.scalar.activation(
                                    p_e[:, 0:W], s_e[:, 0:W],
                                    mybir.ActivationFunctionType.Exp)
                                nc.scalar.activation(
                                    p_o[:, 0:W], s_o[:, 0:W],
                                    mybir.ActivationFunctionType.Exp)
                                for ci, kk in enumerate(chunks):
                                    last = (g == n_g - 1 and ci == len(chunks) - 1)
                                    for par, p_sb in ((0, p_e), (1, p_o)):
                                        # both heads share one PSUM bank: only the
                                        # very first matmul may clear has_written
                                        # (start=True clears the WHOLE bank); the
                                        # odd head's first chunk relies on
                                        # overwrite-where-clear instead.
                                        first = (g == 0 and ci == 0 and par == 0)
                                        nc.tensor.matmul(
                                            o_ps[0:VW, 256 * par:256 * (par + 1)],
                                            vtf[kk][:, VW * (2 * t + par):VW * (2 * t + par) + VW],
                                            p_sb[:, 256 * ci:256 * (ci + 1)],
                                            start=first, stop=last,
                                            skip_group_check=True,
                                        )
                            # normalize both heads of the pair for this stripe
                            dn = dn_ring[(oi - 1) % 2]
                            rc = rc_ring[(oi - 1) % 2]
                            nc.vector.tensor_copy(dn[:], o_ps[64:65, :])
                            nc.tensor.matmul(o_ps[64:128, 0:256], ones_sb[:],
                                             dn[0:1, 0:256], start=True, stop=True)
                            nc.tensor.matmul(o_ps[64:128, 256:512], ones_sb[:],
                                             dn[0:1, 256:512], start=True, stop=True)
                            nc.vector.reciprocal(rc[:], o_ps[64:128, :])
                            for par in (0, 1):
                                nc.vector.tensor_mul(
                                    attnT_sb[t][64 * par:64 * par + 64,
                                                STR * sl:STR * (sl + 1)],
                                    o_ps[0:64, 256 * par:256 * (par + 1)],
                                    rc[:, 256 * par:256 * (par + 1)],
                                )

                for core in range(N_CORES):
                    with tc.If(pid == core):
                        attention_body(core)

            # ---------------- phase 3: output projection --------------------
            with (
                tc.tile_pool(name="outp", bufs=2) as opool,
                tc.tile_pool(name="ps_po", bufs=2, space="PSUM") as ps_po,
            ):
                for qc in range(4):
                    ps = ps_po.tile([128, E], F32, tag="po")
                    for ec in range(6):
                        nc.tensor.matmul(
                            ps[:, 0:512],
                            attnT_sb[ec][:, 128 * qc:128 * (qc + 1)],
                            woutT_sb[ec][:, 0:512],
                            start=(ec == 0), stop=(ec == 5),
                        )
                        nc.tensor.matmul(
                            ps[:, 512:768],
                            attnT_sb[ec][:, 128 * qc:128 * (qc + 1)],
                            woutT_sb[ec][:, 512:768],
                            start=(ec == 0), stop=(ec == 5),
                        )
                    o_sb = opool.tile([128, E], F32, tag="osb")
                    nc.vector.tensor_copy(o_sb[:], ps[:])
                    nc.sync.dma_start(out_own[128 * qc:128 * (qc + 1), :], o_sb[:])

    nc.compile()
    return nc


_cache_lock = threading.Lock()
_cached_nc = None


def _get_program():
    global _cached_nc
    with _cache_lock:
        if _cached_nc is None:
            _cached_nc = build_program()
    return _cached_nc


def _host_fallback(x, mask, w_qkv, b_qkv, w_out, b_out):
    x2 = x.reshape(S, E).astype(np.float64)
    qkv = x2 @ w_qkv.T.astype(np.float64) + b_qkv
    q, k, v = np.split(qkv, 3, axis=-1)
    q = q.reshape(S, H, DH).transpose(1, 0, 2)
    k = k.reshape(S, H, DH).transpose(1, 0, 2)
    v = v.reshape(S, H, DH).transpose(1, 0, 2)
    out = np.empty((H, S, DH))
    for h in range(H):
        sc = q[h] @ k[h].T
        sc = np.where(mask, sc, -np.inf)
        sc = sc - sc.max(axis=-1, keepdims=True)
        p = np.exp(sc)
        p /= p.sum(axis=-1, keepdims=True)
        out[h] = p @ v[h]
    attn = out.transpose(1, 0, 2).reshape(S, E)
    res = attn @ w_out.T.astype(np.float64) + b_out
    return res.reshape(1, S, E).astype(np.float32)


def kernel(x, mask, w_qkv, b_qkv, w_out, b_out, _trace=False):
    x = np.asarray(x)
    mask = np.asarray(mask).astype(bool)
    w_qkv = np.asarray(w_qkv, dtype=np.float32)
    b_qkv = np.asarray(b_qkv, dtype=np.float32)
    w_out = np.asarray(w_out, dtype=np.float32)
    b_out = np.asarray(b_out, dtype=np.float32)

    if not np.array_equal(mask, np.tril(np.ones((S, S), dtype=bool))):
        return _host_fallback(x, mask, w_qkv, b_qkv, w_out, b_out)

    x2 = x.reshape(S, E).astype(np.float32)

    wq = np.zeros((SE, E3), dtype=np.float32)
    wq[:E] = w_qkv.T
    wq[E] = b_qkv
    wq_bf = wq.astype(NP_BF16)

    woutT_np = np.ascontiguousarray(w_out.T).astype(NP_BF16)

    xT_full = np.zeros((SE, S), dtype=np.float32)
    xT_full[:E] = x2.T
    xT_full[E] = 1.0
    xT_bf = xT_full.astype(NP_BF16)

    in_maps = []
    for i in range(N_CORES):
        s0, s1 = STRIPES[i]
        cols = np.r_[STR * s0:STR * (s0 + 1), STR * s1:STR * (s1 + 1)]
        # dbias: per stripe-slot, bias for the last (diagonal) key group
        db = np.zeros((2, 128, 1024), dtype=np.float32)
        for sl, s in enumerate((s0, s1)):
            n_ch = 2 * (s + 1)
            gl = ((n_ch + 3) // 4) - 1
            chunks = list(range(4 * gl, n_ch))
            qrows = slice(STR * s, STR * (s + 1))
            for ci, kk in enumerate(chunks):
                msub = mask[qrows, 128 * kk:128 * (kk + 1)]       # [256 q, 128 k]
                db[sl, :, 256 * ci:256 * (ci + 1)] = np.where(msub.T, 0.0, NEG)
        in_maps.append({
            "xT": np.ascontiguousarray(xT_bf[:, cols]),
            "wqkvT": wq_bf,
            "woutT": woutT_np,
            "dbias": db.astype(NP_BF16),
        })

    nc = _get_program()
    res = run_bass_kernel_spmd(nc, in_maps, core_ids=list(range(N_CORES)),
                               trace=_trace)

    out = np.empty((S, E), dtype=np.float32)
    for i in range(N_CORES):
        s0, s1 = STRIPES[i]
        r = res.results[i]["out_own"]
        out[STR * s0:STR * (s0 + 1)] = r[0:STR]
        out[STR * s1:STR * (s1 + 1)] = r[STR:2 * STR]
    out = out + b_out[None, :]
    result = out.reshape(1, S, E).astype(np.float32)
    if _trace:
        return result, res
    return result


# revision 9
# speedup vs baseline: 1.6109x; 1.1882x over previous
"""Trainium2 Bass kernel for nn_Attention_12403865551261.

Causal multi-head attention (B=1, S=4096, E=768, H=12, Dh=64, no 1/sqrt(dh)
scaling) sharded over 8 NeuronCores.

Sharding: queries are row-sharded in a causal-balanced zigzag — core i owns
the two 256-row stripes {i, 15-i} of the 16-stripe sequence, so every core
does the same total attention work. Each core projects Q/K/V for its own 512
rows (all projections produced directly in transposed layout so no on-device
transposes are needed), K^T and V are AllGathered across the 8 cores in
bf16, then each core runs causal softmax(Q K^T) V for its own rows (only
the lower-triangle key blocks) and the output projection.

Per-core causal structure (different kv extents per stripe) is expressed as
8 compute-only branch bodies on tc.If(partition_id == i); all DMAs and the
projection/collective/output phases are uniform across cores.

Layout trick: scores are computed transposed, S_T[k, q], with dh on the
contraction dim; softmax denominators come free from a ones-column appended
to V (row 64 of the attention-output accumulator). Attention output is
produced directly as attnT[e, q], which is exactly the lhsT the output
projection wants. Score matmuls contract over only 64 partitions, so the
two heads of a pair are issued at partition offsets 0/64 — the PE runs them
concurrently in different row-groups (and the full-width activity keeps the
HAM clock-gate warm).

Mask handling: the causal structure is static; only each stripe's last
(diagonal) key group gets an additive bias tile (0 / -30000, host data).
exp(s - 30000) == 0.0 exactly in f32. If the mask input is not causal, a
host fallback computes the reference directly.
"""
import sys
import threading

sys.path.insert(0, "/opt/trn_rl_repo")

import numpy as np
import ml_dtypes

import concourse.bacc as bacc
import concourse.tile as tile
import concourse.mybir as mybir
from concourse.bass_utils import run_bass_kernel_spmd

BF16 = mybir.dt.bfloat16
F32 = mybir.dt.float32
NP_BF16 = ml_dtypes.bfloat16

N_CORES = 8
S = 4096
E = 768
H = 12
DH = 64
E3 = 3 * E          # 2304
SE = 896            # contraction dim: 768 weights + 1 bias row + pad -> 7*128
KC = SE // 128      # 7 contraction chunks
ROWS = S // N_CORES  # 512 own rows per core
STR = 256           # stripe height
NEG = -30000.0
VW = 65             # V columns per head incl. ones-column

# core i owns stripes (i, 15-i); stripe s covers global rows [256s, 256s+256)
STRIPES = [(i, 15 - i) for i in range(N_CORES)]


def stripe_owner(j):
    """global stripe j -> (core, slot)"""
    c = min(j, 15 - j)
    return c, (0 if j < 8 else 1)


def build_program():
    nc = bacc.Bacc("TRN2", target_bir_lowering=False, debug=False,
                   num_devices=N_CORES)

    xT = nc.dram_tensor("xT", [SE, ROWS], BF16, kind="ExternalInput")
    wqkvT = nc.dram_tensor("wqkvT", [SE, E3], BF16, kind="ExternalInput")
    woutT = nc.dram_tensor("woutT", [E, E], BF16, kind="ExternalInput")
    dbias = nc.dram_tensor("dbias", [2, 128, 1024], BF16, kind="ExternalInput")
    out_own = nc.dram_tensor("out_own", [ROWS, E], F32, kind="ExternalOutput")

    with tile.TileContext(nc) as tc:
        with (
            tc.tile_pool(name="const", bufs=1) as cpool,
            tc.tile_pool(name="dram", bufs=1, space="DRAM") as dram,
        ):
            # block-diagonal Q per (pair, stripe-slot): [Qe|0 ; 0|Qo]
            qd_sb = [[cpool.tile([128, 512], BF16, name=f"qd{t}_{sl}")
                      for sl in range(2)] for t in range(6)]
            attnT_sb = [cpool.tile([128, ROWS], BF16, name=f"at{t}") for t in range(6)]
            woutT_sb = [cpool.tile([128, E], BF16, name=f"wo{t}") for t in range(6)]
            ones_sb = cpool.tile([1, 64], F32, name="ones")
            nc.vector.memset(ones_sb[:], 1.0)
            for t in range(6):
                nc.sync.dma_start(woutT_sb[t][:], woutT[128 * t:128 * (t + 1), :])
            db_sb = [cpool.tile([128, 1024], BF16, name=f"db{sl}") for sl in range(2)]
            for sl in range(2):
                nc.sync.dma_start(db_sb[sl][:], dbias[sl, :, :])

            kt_contrib = dram.tile([E, ROWS], BF16)
            v_contrib = dram.tile([ROWS, H * VW], BF16)
            kt_g = dram.tile([N_CORES * E, ROWS], BF16, addr_space="Shared")
            v_g = dram.tile([S, H * VW], BF16, addr_space="Shared")

            # ---------------- phase 1: QKV projection (own rows) -----------
            with (
                tc.tile_pool(name="proj", bufs=1) as ppool,
                tc.tile_pool(name="pq", bufs=2, space="PSUM") as pq,
                tc.tile_pool(name="pv", bufs=2, space="PSUM") as pv,
            ):
                w_sb = [ppool.tile([128, E3], BF16, name=f"w{t}") for t in range(KC)]
                x_sb = [ppool.tile([128, ROWS], BF16, name=f"x{t}") for t in range(KC)]
                for t in range(KC):
                    nc.sync.dma_start(w_sb[t][:], wqkvT[128 * t:128 * (t + 1), :])
                    nc.sync.dma_start(x_sb[t][:], xT[128 * t:128 * (t + 1), :])

                kt_own = [ppool.tile([128, ROWS], BF16, name=f"kt{t}") for t in range(6)]
                v_own = [ppool.tile([128, H * VW], BF16, name=f"v{t}") for t in range(4)]

                for oc in range(12):
                    ps = pq.tile([128, ROWS], F32, tag="pq")
                    for kc in range(KC):
                        nc.tensor.matmul(
                            ps[:],
                            w_sb[kc][:, 128 * oc:128 * (oc + 1)],
                            x_sb[kc][:],
                            start=(kc == 0), stop=(kc == KC - 1),
                        )
                    if oc < 6:
                        for sl in range(2):
                            qd = qd_sb[oc][sl]
                            nc.vector.memset(qd[:], 0.0)
                            nc.vector.tensor_copy(
                                qd[0:64, 0:256], ps[0:64, STR * sl:STR * (sl + 1)])
                            nc.vector.tensor_copy(
                                qd[64:128, 256:512], ps[64:128, STR * sl:STR * (sl + 1)])
                    else:
                        nc.vector.tensor_copy(kt_own[oc - 6][:], ps[:])

                for sc in range(4):
                    ps = pv.tile([128, E], F32, tag="pv")
                    for kc in range(KC):
                        nc.tensor.matmul(
                            ps[:, 0:512],
                            x_sb[kc][:, 128 * sc:128 * (sc + 1)],
                            w_sb[kc][:, 2 * E:2 * E + 512],
                            start=(kc == 0), stop=(kc == KC - 1),
                        )
                        nc.tensor.matmul(
                            ps[:, 512:768],
                            x_sb[kc][:, 128 * sc:128 * (sc + 1)],
                            w_sb[kc][:, 2 * E + 512:3 * E],
                            start=(kc == 0), stop=(kc == KC - 1),
                        )
                    nc.vector.memset(v_own[sc][:], 1.0)
                    for h in range(H):
                        nc.vector.tensor_copy(
                            v_own[sc][:, VW * h:VW * h + DH],
                            ps[:, DH * h:DH * (h + 1)],
                        )

                for t in range(6):
                    nc.sync.dma_start(kt_contrib[128 * t:128 * (t + 1), :], kt_own[t][:])
                for sc in range(4):
                    nc.sync.dma_start(v_contrib[128 * sc:128 * (sc + 1), :], v_own[sc][:])

                nc.gpsimd.collective_compute(
                    "AllGather", mybir.AluOpType.bypass,
                    replica_groups=[list(range(N_CORES))],
                    ins=[kt_contrib[:]], outs=[kt_g[:]],
                )
                nc.gpsimd.collective_compute(
                    "AllGather", mybir.AluOpType.bypass,
                    replica_groups=[list(range(N_CORES))],
                    ins=[v_contrib[:]], outs=[v_g[:]],
                )

            # ---------------- phase 2: attention ---------------------------
            with (
                tc.tile_pool(name="attn", bufs=1) as apool,
                tc.tile_pool(name="ps_s", bufs=1, space="PSUM") as ps_s,
                tc.tile_pool(name="ps_o", bufs=1, space="PSUM") as ps_o,
            ):
                # gathered K^T -> global-k layout: tile t holds heads 2t,2t+1
                ktf = [apool.tile([128, S], BF16, name=f"ktf{t}") for t in range(6)]
                vtf = [apool.tile([128, H * VW], BF16, name=f"vtf{kk}") for kk in range(32)]
                for j in range(16):              # global stripe j, low keys first
                    c, sl = stripe_owner(j)
                    for t in range(6):
                        nc.sync.dma_start(
                            ktf[t][:, STR * j:STR * (j + 1)],
                            kt_g[E * c + 128 * t:E * c + 128 * (t + 1),
                                 STR * sl:STR * (sl + 1)],
                        )
                    for half in range(2):
                        kk = 2 * j + half
                        src_row = ROWS * c + STR * sl + 128 * half
                        nc.sync.dma_start(vtf[kk][:], v_g[src_row:src_row + 128, :])

                # shared compute rings (only one branch executes per core)
                s_ring = [ps_s.tile([128, 1024], F32, name=f"sr{j}", tag=f"sr{j}")
                          for j in range(3)]
                o_ring = [ps_o.tile([128, 512], F32, name=f"or{j}", tag=f"or{j}")
                          for j in range(2)]
                p_ring = [apool.tile([128, 1024], BF16, name=f"pr{j}") for j in range(3)]
                dn_ring = [apool.tile([1, 512], F32, name=f"dn{j}") for j in range(2)]
                rc_ring = [apool.tile([64, 512], F32, name=f"rcr{j}") for j in range(2)]

                pid = nc.partition_id()

                def attention_body(core):
                    si = 0  # s/p ring counter
                    oi = 0  # o ring counter
                    for t in range(6):              # head pair (2t, 2t+1)
                        for sl, s in enumerate(STRIPES[core]):
                            n_ch = 2 * (s + 1)      # 128-wide key chunks
                            n_g = n_ch // 2         # groups of 2 chunks
                            o_ps = o_ring[oi % 2]; oi += 1
                            for g in range(n_g):
                                chunks = (2 * g, 2 * g + 1)
                                s_ps = s_ring[si % 3]
                                p_sb = p_ring[si % 3]
                                si += 1
                                for ci, kk in enumerate(chunks):
                                    # one K=128 matmul computes both heads'
                                    # score chunk via block-diagonal Q
                                    nc.tensor.matmul(
                                        s_ps[:, 512 * ci:512 * (ci + 1)],
                                        ktf[t][:, 128 * kk:128 * (kk + 1)],
                                        qd_sb[t][sl][:],
                                        start=True, stop=True,
                                    )
                                if g == n_g - 1:   # diagonal group: mask bias
                                    nc.vector.tensor_add(
                                        s_ps[:], s_ps[:], db_sb[sl][:])
                                nc.scalar.activation(
                                    p_sb[:], s_ps[:],
                                    mybir.ActivationFunctionType.Exp)
                                for ci, kk in enumerate(chunks):
                                    last = (g == n_g - 1 and ci == 1)
                                    for par in (0, 1):
                                        # both heads share one PSUM bank: only
                                        # the very first matmul may clear
                                        # has_written (start clears the WHOLE
                                        # bank); the odd head relies on
                                        # overwrite-where-clear.
                                        first = (g == 0 and ci == 0 and par == 0)
                                        nc.tensor.matmul(
                                            o_ps[0:VW, 256 * par:256 * (par + 1)],
                                            vtf[kk][:, VW * (2 * t + par):VW * (2 * t + par) + VW],
                                            p_sb[:, 512 * ci + 256 * par:512 * ci + 256 * par + 256],
                                            start=first, stop=last,
                                            skip_group_check=True,
                                        )
                            # normalize both heads of the pair for this stripe
                            dn = dn_ring[(oi - 1) % 2]
                            rc = rc_ring[(oi - 1) % 2]
                            nc.vector.tensor_copy(dn[:], o_ps[64:65, :])
                            nc.tensor.matmul(o_ps[64:128, 0:256], ones_sb[:],
                                             dn[0:1, 0:256], start=True, stop=True)
                            nc.tensor.matmul(o_ps[64:128, 256:512], ones_sb[:],
                                             dn[0:1, 256:512], start=True, stop=True)
                            nc.vector.reciprocal(rc[:], o_ps[64:128, :])
                            for par in (0, 1):
                                nc.vector.tensor_mul(
                                    attnT_sb[t][64 * par:64 * par + 64,
                                                STR * sl:STR * (sl + 1)],
                                    o_ps[0:64, 256 * par:256 * (par + 1)],
                                    rc[:, 256 * par:256 * (par + 1)],
                                )

                for core in range(N_CORES):
                    with tc.If(pid == core):
                        attention_body(core)

            # ---------------- phase 3: output projection --------------------
            with (
                tc.tile_pool(name="outp", bufs=2) as opool,
                tc.tile_pool(name="ps_po", bufs=2, space="PSUM") as ps_po,
            ):
                for qc in range(4):
                    ps = ps_po.tile([128, E], F32, tag="po")
                    for ec in range(6):
                        nc.tensor.matmul(
                            ps[:, 0:512],
                            attnT_sb[ec][:, 128 * qc:128 * (qc + 1)],
                            woutT_sb[ec][:, 0:512],
                            start=(ec == 0), stop=(ec == 5),
                        )
                        nc.tensor.matmul(
                            ps[:, 512:768],
                            attnT_sb[ec][:, 128 * qc:128 * (qc + 1)],
                            woutT_sb[ec][:, 512:768],
                            start=(ec == 0), stop=(ec == 5),
                        )
                    o_sb = opool.tile([128, E], F32, tag="osb")
                    nc.vector.tensor_copy(o_sb[:], ps[:])
                    nc.sync.dma_start(out_own[128 * qc:128 * (qc + 1), :], o_sb[:])

    nc.compile()
    return nc


_cache_lock = threading.Lock()
_cached_nc = None


def _get_program():
    global _cached_nc
    with _cache_lock:
        if _cached_nc is None:
            _cached_nc = build_program()
    return _cached_nc


def _host_fallback(x, mask, w_qkv, b_qkv, w_out, b_out):
    x2 = x.reshape(S, E).astype(np.float64)
    qkv = x2 @ w_qkv.T.astype(np.float64) + b_qkv
    q, k, v = np.split(qkv, 3, axis=-1)
    q = q.reshape(S, H, DH).transpose(1, 0, 2)
    k = k.reshape(S, H, DH).transpose(1, 0, 2)
    v = v.reshape(S, H, DH).transpose(1, 0, 2)
    out = np.empty((H, S, DH))
    for h in range(H):
        sc = q[h] @ k[h].T
        sc = np.where(mask, sc, -np.inf)
        sc = sc - sc.max(axis=-1, keepdims=True)
        p = np.exp(sc)
        p /= p.sum(axis=-1, keepdims=True)
        out[h] = p @ v[h]
    attn = out.transpose(1, 0, 2).reshape(S, E)
    res = attn @ w_out.T.astype(np.float64) + b_out
    return res.reshape(1, S, E).astype(np.float32)


def kernel(x, mask, w_qkv, b_qkv, w_out, b_out, _trace=False):
    x = np.asarray(x)
    mask = np.asarray(mask).astype(bool)
    w_qkv = np.asarray(w_qkv, dtype=np.float32)
    b_qkv = np.asarray(b_qkv, dtype=np.float32)
    w_out = np.asarray(w_out, dtype=np.float32)
    b_out = np.asarray(b_out, dtype=np.float32)

    if not np.array_equal(mask, np.tril(np.ones((S, S), dtype=bool))):
        return _host_fallback(x, mask, w_qkv, b_qkv, w_out, b_out)

    x2 = x.reshape(S, E).astype(np.float32)

    wq = np.zeros((SE, E3), dtype=np.float32)
    wq[:E] = w_qkv.T
    wq[E] = b_qkv
    wq_bf = wq.astype(NP_BF16)

    woutT_np = np.ascontiguousarray(w_out.T).astype(NP_BF16)

    xT_full = np.zeros((SE, S), dtype=np.float32)
    xT_full[:E] = x2.T
    xT_full[E] = 1.0
    xT_bf = xT_full.astype(NP_BF16)

    in_maps = []
    for i in range(N_CORES):
        s0, s1 = STRIPES[i]
        cols = np.r_[STR * s0:STR * (s0 + 1), STR * s1:STR * (s1 + 1)]
        # dbias: per stripe-slot, bias for the last (diagonal) 2-chunk group,
        # layout [k, 512*ci + 256*par + q] (duplicated across the head pair)
        db = np.zeros((2, 128, 1024), dtype=np.float32)
        for sl, s in enumerate((s0, s1)):
            qrows = slice(STR * s, STR * (s + 1))
            for ci in range(2):
                kk = 2 * s + ci
                msub = mask[qrows, 128 * kk:128 * (kk + 1)]       # [256 q, 128 k]
                bias = np.where(msub.T, 0.0, NEG)                 # [128 k, 256 q]
                db[sl, :, 512 * ci:512 * ci + 256] = bias
                db[sl, :, 512 * ci + 256:512 * ci + 512] = bias
        in_maps.append({
            "xT": np.ascontiguousarray(xT_bf[:, cols]),
            "wqkvT": wq_bf,
            "woutT": woutT_np,
            "dbias": db.astype(NP_BF16),
        })

    nc = _get_program()
    res = run_bass_kernel_spmd(nc, in_maps, core_ids=list(range(N_CORES)),
                               trace=_trace)

    out = np.empty((S, E), dtype=np.float32)
    for i in range(N_CORES):
        s0, s1 = STRIPES[i]
        r = res.results[i]["out_own"]
        out[STR * s0:STR * (s0 + 1)] = r[0:STR]
        out[STR * s1:STR * (s1 + 1)] = r[STR:2 * STR]
    out = out + b_out[None, :]
    result = out.reshape(1, S, E).astype(np.float32)
    if _trace:
        return result, res
    return result


# revision 11
# speedup vs baseline: 1.6568x; 1.0285x over previous
"""Trainium2 Bass kernel for nn_Attention_12403865551261.

Causal multi-head attention (B=1, S=4096, E=768, H=12, Dh=64, no 1/sqrt(dh)
scaling) sharded over 8 NeuronCores.

Sharding: queries are row-sharded in a causal-balanced zigzag — core i owns
the two 256-row stripes {i, 15-i} of the 16-stripe sequence, so every core
does the same total attention work. Each core projects Q/K/V for its own 512
rows (all projections produced directly in transposed layout so no on-device
transposes are needed), K^T and V are AllGathered across the 8 cores in
bf16, then each core runs causal softmax(Q K^T) V for its own rows (only
the lower-triangle key blocks) and the output projection.

Per-core causal structure (different kv extents per stripe) is expressed as
8 compute-only branch bodies on tc.If(partition_id == i); all DMAs and the
projection/collective/output phases are uniform across cores.

Layout trick: scores are computed transposed, S_T[k, q], with dh on the
contraction dim; softmax denominators come free from a ones-column appended
to V (row 64 of the attention-output accumulator). Attention output is
produced directly as attnT[e, q], which is exactly the lhsT the output
projection wants. Score matmuls contract over only 64 partitions, so the
two heads of a pair are issued at partition offsets 0/64 — the PE runs them
concurrently in different row-groups (and the full-width activity keeps the
HAM clock-gate warm).

Mask handling: the causal structure is static; only each stripe's last
(diagonal) key group gets an additive bias tile (0 / -30000, host data).
exp(s - 30000) == 0.0 exactly in f32. If the mask input is not causal, a
host fallback computes the reference directly.
"""
import sys
import threading

sys.path.insert(0, "/opt/trn_rl_repo")

import numpy as np
import ml_dtypes

import concourse.bacc as bacc
import concourse.tile as tile
import concourse.mybir as mybir
from concourse.bass_utils import run_bass_kernel_spmd

BF16 = mybir.dt.bfloat16
F32 = mybir.dt.float32
NP_BF16 = ml_dtypes.bfloat16

N_CORES = 8
S = 4096
E = 768
H = 12
DH = 64
E3 = 3 * E          # 2304
SE = 896            # contraction dim: 768 weights + 1 bias row + pad -> 7*128
KC = SE // 128      # 7 contraction chunks
ROWS = S // N_CORES  # 512 own rows per core
STR = 256           # stripe height
NEG = -30000.0
VW = 65             # V columns per head incl. ones-column

# core i owns stripes (i, 15-i); stripe s covers global rows [256s, 256s+256)
STRIPES = [(i, 15 - i) for i in range(N_CORES)]


def stripe_owner(j):
    """global stripe j -> (core, slot)"""
    c = min(j, 15 - j)
    return c, (0 if j < 8 else 1)


def build_program():
    nc = bacc.Bacc("TRN2", target_bir_lowering=False, debug=False,
                   num_devices=N_CORES)

    xT = nc.dram_tensor("xT", [SE, ROWS], BF16, kind="ExternalInput")
    wqkvT = nc.dram_tensor("wqkvT", [SE, E3], BF16, kind="ExternalInput")
    woutT = nc.dram_tensor("woutT", [E, E], BF16, kind="ExternalInput")
    dbias = nc.dram_tensor("dbias", [2, 128, 1024], BF16, kind="ExternalInput")
    out_own = nc.dram_tensor("out_own", [ROWS, E], F32, kind="ExternalOutput")

    with tile.TileContext(nc) as tc:
        with (
            tc.tile_pool(name="const", bufs=1) as cpool,
            tc.tile_pool(name="dram", bufs=1, space="DRAM") as dram,
        ):
            # block-diagonal Q per (pair, stripe-slot): [Qe|0 ; 0|Qo]
            qd_sb = [[cpool.tile([128, 512], BF16, name=f"qd{t}_{sl}")
                      for sl in range(2)] for t in range(6)]
            attnT_sb = [cpool.tile([128, ROWS], BF16, name=f"at{t}") for t in range(6)]
            woutT_sb = [cpool.tile([128, E], BF16, name=f"wo{t}") for t in range(6)]
            ones_sb = cpool.tile([1, 64], F32, name="ones")
            nc.vector.memset(ones_sb[:], 1.0)
            for t in range(6):
                nc.sync.dma_start(woutT_sb[t][:], woutT[128 * t:128 * (t + 1), :])
            db_sb = [cpool.tile([128, 1024], BF16, name=f"db{sl}") for sl in range(2)]
            for sl in range(2):
                nc.sync.dma_start(db_sb[sl][:], dbias[sl, :, :])

            kt_contrib = dram.tile([E, ROWS], BF16)
            v_contrib = dram.tile([ROWS, H * VW], BF16)
            kt_g = dram.tile([N_CORES * E, ROWS], BF16, addr_space="Shared")
            v_g = dram.tile([S, H * VW], BF16, addr_space="Shared")

            # ---------------- phase 1: QKV projection (own rows) -----------
            with (
                tc.tile_pool(name="proj", bufs=1) as ppool,
                tc.tile_pool(name="pq", bufs=2, space="PSUM") as pq,
                tc.tile_pool(name="pv", bufs=2, space="PSUM") as pv,
            ):
                w_sb = [ppool.tile([128, E3], BF16, name=f"w{t}") for t in range(KC)]
                x_sb = [ppool.tile([128, ROWS], BF16, name=f"x{t}") for t in range(KC)]
                for t in range(KC):
                    nc.sync.dma_start(w_sb[t][:], wqkvT[128 * t:128 * (t + 1), :])
                    nc.sync.dma_start(x_sb[t][:], xT[128 * t:128 * (t + 1), :])

                kt_own = [cpool.tile([128, ROWS], BF16, name=f"kt{t}") for t in range(6)]
                v_own = [cpool.tile([128, H * VW], BF16, name=f"v{t}") for t in range(4)]

                for oc in range(12):
                    ps = pq.tile([128, ROWS], F32, tag="pq")
                    for kc in range(KC):
                        nc.tensor.matmul(
                            ps[:],
                            w_sb[kc][:, 128 * oc:128 * (oc + 1)],
                            x_sb[kc][:],
                            start=(kc == 0), stop=(kc == KC - 1),
                        )
                    if oc < 6:
                        for sl in range(2):
                            qd = qd_sb[oc][sl]
                            nc.vector.memset(qd[:], 0.0)
                            nc.vector.tensor_copy(
                                qd[0:64, 0:256], ps[0:64, STR * sl:STR * (sl + 1)])
                            nc.vector.tensor_copy(
                                qd[64:128, 256:512], ps[64:128, STR * sl:STR * (sl + 1)])
                    else:
                        nc.vector.tensor_copy(kt_own[oc - 6][:], ps[:])

                for sc in range(4):
                    ps = pv.tile([128, E], F32, tag="pv")
                    for kc in range(KC):
                        nc.tensor.matmul(
                            ps[:, 0:512],
                            x_sb[kc][:, 128 * sc:128 * (sc + 1)],
                            w_sb[kc][:, 2 * E:2 * E + 512],
                            start=(kc == 0), stop=(kc == KC - 1),
                        )
                        nc.tensor.matmul(
                            ps[:, 512:768],
                            x_sb[kc][:, 128 * sc:128 * (sc + 1)],
                            w_sb[kc][:, 2 * E + 512:3 * E],
                            start=(kc == 0), stop=(kc == KC - 1),
                        )
                    nc.vector.memset(v_own[sc][:], 1.0)
                    for h in range(H):
                        nc.vector.tensor_copy(
                            v_own[sc][:, VW * h:VW * h + DH],
                            ps[:, DH * h:DH * (h + 1)],
                        )

                for t in range(6):
                    nc.sync.dma_start(kt_contrib[128 * t:128 * (t + 1), :], kt_own[t][:])
                for sc in range(4):
                    nc.sync.dma_start(v_contrib[128 * sc:128 * (sc + 1), :], v_own[sc][:])

                nc.gpsimd.collective_compute(
                    "AllGather", mybir.AluOpType.bypass,
                    replica_groups=[list(range(N_CORES))],
                    ins=[kt_contrib[:]], outs=[kt_g[:]],
                )
                nc.gpsimd.collective_compute(
                    "AllGather", mybir.AluOpType.bypass,
                    replica_groups=[list(range(N_CORES))],
                    ins=[v_contrib[:]], outs=[v_g[:]],
                )

            # ---------------- phase 2: attention ---------------------------
            with (
                tc.tile_pool(name="attn", bufs=1) as apool,
                tc.tile_pool(name="ps_s", bufs=1, space="PSUM") as ps_s,
                tc.tile_pool(name="ps_o", bufs=1, space="PSUM") as ps_o,
            ):
                ktf = [apool.tile([128, S], BF16, name=f"ktf{t}") for t in range(6)]
                vtf = [apool.tile([128, H * VW], BF16, name=f"vtf{kk}") for kk in range(32)]
                s_ring = [ps_s.tile([128, 1024], F32, name=f"sr{j}", tag=f"sr{j}")
                          for j in range(3)]
                o_ring = [ps_o.tile([128, 512], F32, name=f"or{j}", tag=f"or{j}")
                          for j in range(2)]
                p_ring = [apool.tile([128, 1024], BF16, name=f"pr{j}") for j in range(3)]
                rc_ring = [apool.tile([64, 512], F32, name=f"rcr{j}") for j in range(2)]
                dn_ring = [apool.tile([1, 512], F32, name=f"dn{j}") for j in range(2)]
                osum_ring = [apool.tile([VW, 512], F32, name=f"os{j}") for j in range(2)]
                o_diag = [[apool.tile([VW, 512], F32, name=f"od{t}_{sl}")
                           for sl in range(2)] for t in range(6)]
                ob_ring = [apool.tile([128, E], F32, name=f"ob{j}") for j in range(2)]

                # --- diagonal groups from OWN K/V: uniform across cores, so it
                # runs outside the branches and overlaps the AllGather -------
                dsi = 0
                doi = 0
                for t in range(6):
                    for sl in range(2):
                        s_ps = s_ring[dsi % 3]
                        p_sb = p_ring[dsi % 3]
                        dsi += 1
                        for ci in range(2):
                            nc.tensor.matmul(
                                s_ps[:, 512 * ci:512 * (ci + 1)],
                                kt_own[t][:, STR * sl + 128 * ci:STR * sl + 128 * (ci + 1)],
                                qd_sb[t][sl][:],
                                start=True, stop=True,
                            )
                        nc.vector.tensor_add(s_ps[:], s_ps[:], db_sb[sl][:])
                        nc.scalar.activation(p_sb[:], s_ps[:],
                                             mybir.ActivationFunctionType.Exp)
                        o_ps = o_ring[doi % 2]
                        doi += 1
                        for ci in range(2):
                            for par in (0, 1):
                                nc.tensor.matmul(
                                    o_ps[0:VW, 256 * par:256 * (par + 1)],
                                    v_own[2 * sl + ci][:, VW * (2 * t + par):VW * (2 * t + par) + VW],
                                    p_sb[:, 512 * ci + 256 * par:512 * ci + 256 * par + 256],
                                    start=(ci == 0 and par == 0),
                                    stop=(ci == 1 and par == 1),
                                    skip_group_check=True,
                                )
                        nc.vector.tensor_copy(o_diag[t][sl][:], o_ps[0:VW, :])

                pid = nc.partition_id()

                def attention_body(core):
                    si = 0
                    oi = 0
                    # gather loads inside the branch so they overlap compute
                    for j in range(16):              # global stripe j, low keys first
                        c, sl = stripe_owner(j)
                        for t in range(6):
                            nc.sync.dma_start(
                                ktf[t][:, STR * j:STR * (j + 1)],
                                kt_g[E * c + 128 * t:E * c + 128 * (t + 1),
                                     STR * sl:STR * (sl + 1)],
                            )
                        for half in range(2):
                            kk = 2 * j + half
                            src_row = ROWS * c + STR * sl + 128 * half
                            nc.sync.dma_start(vtf[kk][:], v_g[src_row:src_row + 128, :])

                    for sl in range(2):
                        s = STRIPES[core][sl]
                        for t in range(6):
                            # main (non-diagonal) key groups from gathered K/V
                            o_ps = o_ring[oi % 2]
                            oi += 1
                            for g in range(s):
                                chunks = (2 * g, 2 * g + 1)
                                s_ps = s_ring[si % 3]
                                p_sb = p_ring[si % 3]
                                si += 1
                                for ci, kk in enumerate(chunks):
                                    nc.tensor.matmul(
                                        s_ps[:, 512 * ci:512 * (ci + 1)],
                                        ktf[t][:, 128 * kk:128 * (kk + 1)],
                                        qd_sb[t][sl][:],
                                        start=True, stop=True,
                                    )
                                nc.scalar.activation(p_sb[:], s_ps[:],
                                                     mybir.ActivationFunctionType.Exp)
                                for ci, kk in enumerate(chunks):
                                    for par in (0, 1):
                                        # o_ps bank is shared by the head pair:
                                        # only the very first matmul clears
                                        # has_written
                                        nc.tensor.matmul(
                                            o_ps[0:VW, 256 * par:256 * (par + 1)],
                                            vtf[kk][:, VW * (2 * t + par):VW * (2 * t + par) + VW],
                                            p_sb[:, 512 * ci + 256 * par:512 * ci + 256 * par + 256],
                                            start=(g == 0 and ci == 0 and par == 0),
                                            stop=(g == s - 1 and ci == 1 and par == 1),
                                            skip_group_check=True,
                                        )
                            # combine with diagonal partial and normalize
                            if s == 0:
                                o_sum = o_diag[t][sl]
                            else:
                                o_sum = osum_ring[oi % 2]
                                nc.vector.tensor_add(o_sum[:], o_ps[0:VW, :],
                                                     o_diag[t][sl][:])
                            rc = rc_ring[oi % 2]
                            dn = dn_ring[oi % 2]
                            nc.vector.tensor_copy(dn[:], o_sum[64:65, :])
                            nc.tensor.matmul(o_ps[64:128, 0:256], ones_sb[:],
                                             dn[0:1, 0:256], start=True, stop=True)
                            nc.tensor.matmul(o_ps[64:128, 256:512], ones_sb[:],
                                             dn[0:1, 256:512], start=True, stop=True)
                            nc.vector.reciprocal(rc[:], o_ps[64:128, :])
                            for par in (0, 1):
                                nc.vector.tensor_mul(
                                    attnT_sb[t][64 * par:64 * par + 64,
                                                STR * sl:STR * (sl + 1)],
                                    o_sum[0:64, 256 * par:256 * (par + 1)],
                                    rc[:, 256 * par:256 * (par + 1)],
                                )

                    # output projection (in-branch: overlaps slot-1 attention)
                    for qc in range(4):
                        o_ps = o_ring[oi % 2]
                        oi += 1
                        o_sb = ob_ring[qc % 2]
                        for ec in range(6):
                            nc.tensor.matmul(
                                o_ps[:, 0:512],
                                attnT_sb[ec][:, 128 * qc:128 * (qc + 1)],
                                woutT_sb[ec][:, 0:512],
                                start=(ec == 0), stop=(ec == 5),
                            )
                        nc.vector.tensor_copy(o_sb[:, 0:512], o_ps[:, 0:512])
                        for ec in range(6):
                            nc.tensor.matmul(
                                o_ps[:, 0:256],
                                attnT_sb[ec][:, 128 * qc:128 * (qc + 1)],
                                woutT_sb[ec][:, 512:768],
                                start=(ec == 0), stop=(ec == 5),
                            )
                        nc.vector.tensor_copy(o_sb[:, 512:768], o_ps[:, 0:256])
                        nc.sync.dma_start(out_own[128 * qc:128 * (qc + 1), :], o_sb[:])

                for core in range(N_CORES):
                    with tc.If(pid == core):
                        attention_body(core)

    nc.compile()
    return nc


_cache_lock = threading.Lock()
_cached_nc = None


def _get_program():
    global _cached_nc
    with _cache_lock:
        if _cached_nc is None:
            _cached_nc = build_program()
    return _cached_nc


def _host_fallback(x, mask, w_qkv, b_qkv, w_out, b_out):
    x2 = x.reshape(S, E).astype(np.float64)
    qkv = x2 @ w_qkv.T.astype(np.float64) + b_qkv
    q, k, v = np.split(qkv, 3, axis=-1)
    q = q.reshape(S, H, DH).transpose(1, 0, 2)
    k = k.reshape(S, H, DH).transpose(1, 0, 2)
    v = v.reshape(S, H, DH).transpose(1, 0, 2)
    out = np.empty((H, S, DH))
    for h in range(H):
        sc = q[h] @ k[h].T
        sc = np.where(mask, sc, -np.inf)
        sc = sc - sc.max(axis=-1, keepdims=True)
        p = np.exp(sc)
        p /= p.sum(axis=-1, keepdims=True)
        out[h] = p @ v[h]
    attn = out.transpose(1, 0, 2).reshape(S, E)
    res = attn @ w_out.T.astype(np.float64) + b_out
    return res.reshape(1, S, E).astype(np.float32)


def kernel(x, mask, w_qkv, b_qkv, w_out, b_out, _trace=False):
    x = np.asarray(x)
    mask = np.asarray(mask).astype(bool)
    w_qkv = np.asarray(w_qkv, dtype=np.float32)
    b_qkv = np.asarray(b_qkv, dtype=np.float32)
    w_out = np.asarray(w_out, dtype=np.float32)
    b_out = np.asarray(b_out, dtype=np.float32)

    if not np.array_equal(mask, np.tril(np.ones((S, S), dtype=bool))):
        return _host_fallback(x, mask, w_qkv, b_qkv, w_out, b_out)

    x2 = x.reshape(S, E).astype(np.float32)

    wq = np.zeros((SE, E3), dtype=np.float32)
    wq[:E] = w_qkv.T
    wq[E] = b_qkv
    wq_bf = wq.astype(NP_BF16)

    woutT_np = np.ascontiguousarray(w_out.T).astype(NP_BF16)

    xT_full = np.zeros((SE, S), dtype=np.float32)
    xT_full[:E] = x2.T
    xT_full[E] = 1.0
    xT_bf = xT_full.astype(NP_BF16)

    in_maps = []
    for i in range(N_CORES):
        s0, s1 = STRIPES[i]
        cols = np.r_[STR * s0:STR * (s0 + 1), STR * s1:STR * (s1 + 1)]
        # dbias: per stripe-slot, bias for the last (diagonal) 2-chunk group,
        # layout [k, 512*ci + 256*par + q] (duplicated across the head pair)
        db = np.zeros((2, 128, 1024), dtype=np.float32)
        for sl, s in enumerate((s0, s1)):
            qrows = slice(STR * s, STR * (s + 1))
            for ci in range(2):
                kk = 2 * s + ci
                msub = mask[qrows, 128 * kk:128 * (kk + 1)]       # [256 q, 128 k]
                bias = np.where(msub.T, 0.0, NEG)                 # [128 k, 256 q]
                db[sl, :, 512 * ci:512 * ci + 256] = bias
                db[sl, :, 512 * ci + 256:512 * ci + 512] = bias
        in_maps.append({
            "xT": np.ascontiguousarray(xT_bf[:, cols]),
            "wqkvT": wq_bf,
            "woutT": woutT_np,
            "dbias": db.astype(NP_BF16),
        })

    nc = _get_program()
    res = run_bass_kernel_spmd(nc, in_maps, core_ids=list(range(N_CORES)),
                               trace=_trace)

    out = np.empty((S, E), dtype=np.float32)
    for i in range(N_CORES):
        s0, s1 = STRIPES[i]
        r = res.results[i]["out_own"]
        out[STR * s0:STR * (s0 + 1)] = r[0:STR]
        out[STR * s1:STR * (s1 + 1)] = r[STR:2 * STR]
    out = out + b_out[None, :]
    result = out.reshape(1, S, E).astype(np.float32)
    if _trace:
        return result, res
    return result


# revision 12
# speedup vs baseline: 1.7407x; 1.0506x over previous
"""Trainium2 Bass kernel for nn_Attention_12403865551261.

Causal multi-head attention (B=1, S=4096, E=768, H=12, Dh=64, no 1/sqrt(dh)
scaling) sharded over 8 NeuronCores.

Sharding: queries are row-sharded in a causal-balanced zigzag — core i owns
the two 256-row stripes {i, 15-i} of the 16-stripe sequence, so every core
does the same total attention work. Each core projects Q/K/V for its own 512
rows (all projections produced directly in transposed layout so no on-device
transposes are needed), K^T and V are AllGathered across the 8 cores in
bf16, then each core runs causal softmax(Q K^T) V for its own rows (only
the lower-triangle key blocks) and the output projection.

Per-core causal structure (different kv extents per stripe) is expressed as
8 compute-only branch bodies on tc.If(partition_id == i); all DMAs and the
projection/collective/output phases are uniform across cores.

Layout trick: scores are computed transposed, S_T[k, q], with dh on the
contraction dim; softmax denominators come free from a ones-column appended
to V (row 64 of the attention-output accumulator). Attention output is
produced directly as attnT[e, q], which is exactly the lhsT the output
projection wants. Score matmuls contract over only 64 partitions, so the
two heads of a pair are issued at partition offsets 0/64 — the PE runs them
concurrently in different row-groups (and the full-width activity keeps the
HAM clock-gate warm).

Mask handling: the causal structure is static; only each stripe's last
(diagonal) key group gets an additive bias tile (0 / -30000, host data).
exp(s - 30000) == 0.0 exactly in f32. If the mask input is not causal, a
host fallback computes the reference directly.
"""
import sys
import threading

sys.path.insert(0, "/opt/trn_rl_repo")

import numpy as np
import ml_dtypes

import concourse.bacc as bacc
import concourse.tile as tile
import concourse.mybir as mybir
from concourse.bass_utils import run_bass_kernel_spmd

BF16 = mybir.dt.bfloat16
F32 = mybir.dt.float32
NP_BF16 = ml_dtypes.bfloat16

N_CORES = 8
S = 4096
E = 768
H = 12
DH = 64
E3 = 3 * E          # 2304
SE = 896            # contraction dim: 768 weights + 1 bias row + pad -> 7*128
KC = SE // 128      # 7 contraction chunks
ROWS = S // N_CORES  # 512 own rows per core
STR = 256           # stripe height
NEG = -30000.0
VW = 65             # V columns per head incl. ones-column

# core i owns stripes (i, 15-i); stripe s covers global rows [256s, 256s+256)
STRIPES = [(i, 15 - i) for i in range(N_CORES)]


def stripe_owner(j):
    """global stripe j -> (core, slot)"""
    c = min(j, 15 - j)
    return c, (0 if j < 8 else 1)


def build_program():
    nc = bacc.Bacc("TRN2", target_bir_lowering=False, debug=False,
                   num_devices=N_CORES)

    xT = nc.dram_tensor("xT", [SE, ROWS], BF16, kind="ExternalInput")
    wqkvT = nc.dram_tensor("wqkvT", [SE, E3], BF16, kind="ExternalInput")
    woutT = nc.dram_tensor("woutT", [E, E], BF16, kind="ExternalInput")
    dbias = nc.dram_tensor("dbias", [2, 128, 1024], BF16, kind="ExternalInput")
    out_own = nc.dram_tensor("out_own", [ROWS, E], F32, kind="ExternalOutput")

    with tile.TileContext(nc) as tc:
        with (
            tc.tile_pool(name="const", bufs=1) as cpool,
            tc.tile_pool(name="dram", bufs=1, space="DRAM") as dram,
        ):
            # block-diagonal Q per (pair, stripe-slot): [Qe|0 ; 0|Qo]
            qd_sb = [[cpool.tile([128, 512], BF16, name=f"qd{t}_{sl}")
                      for sl in range(2)] for t in range(6)]
            attnT_sb = [cpool.tile([128, ROWS], BF16, name=f"at{t}") for t in range(6)]
            woutT_sb = [cpool.tile([128, E], BF16, name=f"wo{t}") for t in range(6)]
            kt_own = [cpool.tile([128, ROWS], BF16, name=f"kt{t}") for t in range(6)]
            v_own = [cpool.tile([128, H * VW], BF16, name=f"v{t}") for t in range(4)]
            ones_sb = cpool.tile([1, 64], F32, name="ones")
            nc.vector.memset(ones_sb[:], 1.0)
            for t in range(6):
                nc.sync.dma_start(woutT_sb[t][:], woutT[128 * t:128 * (t + 1), :])
            db_sb = [cpool.tile([128, 1024], BF16, name=f"db{sl}") for sl in range(2)]
            for sl in range(2):
                nc.sync.dma_start(db_sb[sl][:], dbias[sl, :, :])

            # lo = slot-0 stripes (global stripes 0-7), hi = slot-1 (8-15)
            ktc = [dram.tile([E, STR], BF16, name=f"ktc{h}") for h in range(2)]
            vc = [dram.tile([STR, H * VW], BF16, name=f"vc{h}") for h in range(2)]
            kt_g = [dram.tile([N_CORES * E, STR], BF16, addr_space="Shared",
                              name=f"ktg{h}") for h in range(2)]
            v_g = [dram.tile([N_CORES * STR, H * VW], BF16, addr_space="Shared",
                             name=f"vg{h}") for h in range(2)]

            # ---------------- phase 1: QKV projection (own rows) -----------
            with (
                tc.tile_pool(name="proj", bufs=1) as ppool,
                tc.tile_pool(name="pq", bufs=2, space="PSUM") as pq,
                tc.tile_pool(name="pv", bufs=2, space="PSUM") as pv,
            ):
                w_sb = [ppool.tile([128, E3], BF16, name=f"w{t}") for t in range(KC)]
                x_sb = [ppool.tile([128, ROWS], BF16, name=f"x{t}") for t in range(KC)]
                for t in range(KC):
                    nc.sync.dma_start(w_sb[t][:], wqkvT[128 * t:128 * (t + 1), :])
                    nc.sync.dma_start(x_sb[t][:], xT[128 * t:128 * (t + 1), :])

                # K first so its all-gather starts while Q/V still project
                for oc in range(6, 12):
                    ps = pq.tile([128, ROWS], F32, tag="pq")
                    for kc in range(KC):
                        nc.tensor.matmul(
                            ps[:],
                            w_sb[kc][:, 128 * oc:128 * (oc + 1)],
                            x_sb[kc][:],
                            start=(kc == 0), stop=(kc == KC - 1),
                        )
                    t = oc - 6
                    nc.vector.tensor_copy(kt_own[t][:], ps[:])
                    for h in range(2):
                        nc.sync.dma_start(
                            ktc[h][128 * t:128 * (t + 1), :],
                            kt_own[t][:, STR * h:STR * (h + 1)])
                for h in range(2):
                    nc.gpsimd.collective_compute(
                        "AllGather", mybir.AluOpType.bypass,
                        replica_groups=[list(range(N_CORES))],
                        ins=[ktc[h][:]], outs=[kt_g[h][:]],
                    )

                for oc in range(6):
                    ps = pq.tile([128, ROWS], F32, tag="pq")
                    for kc in range(KC):
                        nc.tensor.matmul(
                            ps[:],
                            w_sb[kc][:, 128 * oc:128 * (oc + 1)],
                            x_sb[kc][:],
                            start=(kc == 0), stop=(kc == KC - 1),
                        )
                    for sl in range(2):
                        qd = qd_sb[oc][sl]
                        nc.vector.memset(qd[:], 0.0)
                        nc.vector.tensor_copy(
                            qd[0:64, 0:256], ps[0:64, STR * sl:STR * (sl + 1)])
                        nc.vector.tensor_copy(
                            qd[64:128, 256:512], ps[64:128, STR * sl:STR * (sl + 1)])

                for sc in range(4):
                    ps = pv.tile([128, E], F32, tag="pv")
                    for kc in range(KC):
                        nc.tensor.matmul(
                            ps[:, 0:512],
                            x_sb[kc][:, 128 * sc:128 * (sc + 1)],
                            w_sb[kc][:, 2 * E:2 * E + 512],
                            start=(kc == 0), stop=(kc == KC - 1),
                        )
                        nc.tensor.matmul(
                            ps[:, 512:768],
                            x_sb[kc][:, 128 * sc:128 * (sc + 1)],
                            w_sb[kc][:, 2 * E + 512:3 * E],
                            start=(kc == 0), stop=(kc == KC - 1),
                        )
                    nc.vector.memset(v_own[sc][:], 1.0)
                    for h in range(H):
                        nc.vector.tensor_copy(
                            v_own[sc][:, VW * h:VW * h + DH],
                            ps[:, DH * h:DH * (h + 1)],
                        )
                    nc.sync.dma_start(vc[sc // 2][128 * (sc % 2):128 * (sc % 2 + 1), :],
                                      v_own[sc][:])
                for h in range(2):
                    nc.gpsimd.collective_compute(
                        "AllGather", mybir.AluOpType.bypass,
                        replica_groups=[list(range(N_CORES))],
                        ins=[vc[h][:]], outs=[v_g[h][:]],
                    )

            # ---------------- phase 2: attention ---------------------------
            with (
                tc.tile_pool(name="attn", bufs=1) as apool,
                tc.tile_pool(name="ps_s", bufs=1, space="PSUM") as ps_s,
                tc.tile_pool(name="ps_o", bufs=1, space="PSUM") as ps_o,
            ):
                ktf = [apool.tile([128, S], BF16, name=f"ktf{t}") for t in range(6)]
                vtf = [apool.tile([128, H * VW], BF16, name=f"vtf{kk}") for kk in range(32)]
                s_ring = [ps_s.tile([128, 1024], F32, name=f"sr{j}", tag=f"sr{j}")
                          for j in range(2)]
                o_ring = [ps_o.tile([128, 512], F32, name=f"or{j}", tag=f"or{j}")
                          for j in range(3)]
                p_ring = [apool.tile([128, 1024], BF16, name=f"pr{j}") for j in range(2)]
                rc_ring = [apool.tile([64, 512], F32, name=f"rcr{j}") for j in range(2)]
                dn_ring = [apool.tile([1, 512], F32, name=f"dn{j}") for j in range(2)]
                osum_ring = [apool.tile([VW, 512], F32, name=f"os{j}") for j in range(2)]
                o_diag = [[apool.tile([VW, 512], F32, name=f"od{t}_{sl}")
                           for sl in range(2)] for t in range(6)]
                ob_ring = [apool.tile([128, E], F32, name=f"ob{j}") for j in range(2)]

                # --- diagonal groups from OWN K/V: uniform across cores, so it
                # runs outside the branches and overlaps the AllGathers ------
                dsi = 0
                doi = 0
                for t in range(6):
                    for sl in range(2):
                        s_ps = s_ring[dsi % 2]
                        p_sb = p_ring[dsi % 2]
                        dsi += 1
                        for ci in range(2):
                            nc.tensor.matmul(
                                s_ps[:, 512 * ci:512 * (ci + 1)],
                                kt_own[t][:, STR * sl + 128 * ci:STR * sl + 128 * (ci + 1)],
                                qd_sb[t][sl][:],
                                start=True, stop=True,
                            )
                        nc.vector.tensor_add(s_ps[:], s_ps[:], db_sb[sl][:])
                        nc.scalar.activation(p_sb[:], s_ps[:],
                                             mybir.ActivationFunctionType.Exp)
                        o_ps = o_ring[doi % 3]
                        doi += 1
                        for ci in range(2):
                            for par in (0, 1):
                                nc.tensor.matmul(
                                    o_ps[0:VW, 256 * par:256 * (par + 1)],
                                    v_own[2 * sl + ci][:, VW * (2 * t + par):VW * (2 * t + par) + VW],
                                    p_sb[:, 512 * ci + 256 * par:512 * ci + 256 * par + 256],
                                    start=(ci == 0 and par == 0),
                                    stop=(ci == 1 and par == 1),
                                    skip_group_check=True,
                                )
                        nc.vector.tensor_copy(o_diag[t][sl][:], o_ps[0:VW, :])

                pid = nc.partition_id()

                def attention_body(core):
                    si = 0
                    oi = 0
                    ni = 0
                    # gather loads inside the branch so they overlap compute
                    for j in range(16):              # global stripe j, low keys first
                        h, rank = (0, j) if j < 8 else (1, 15 - j)
                        for t in range(6):
                            nc.sync.dma_start(
                                ktf[t][:, STR * j:STR * (j + 1)],
                                kt_g[h][E * rank + 128 * t:E * rank + 128 * (t + 1), :],
                            )
                        for half in range(2):
                            nc.sync.dma_start(
                                vtf[2 * j + half][:],
                                v_g[h][STR * rank + 128 * half:STR * rank + 128 * (half + 1), :])

                    # big stripe first: denser PE work, and the small stripe's
                    # latency-bound normalizations overlap the output proj
                    for sl in (1, 0):
                        s = STRIPES[core][sl]
                        for t in range(6):
                            o_ps = o_ring[oi % 3]
                            oi += 1
                            for g in range(s):
                                chunks = (2 * g, 2 * g + 1)
                                s_ps = s_ring[si % 2]
                                p_sb = p_ring[si % 2]
                                si += 1
                                for ci, kk in enumerate(chunks):
                                    nc.tensor.matmul(
                                        s_ps[:, 512 * ci:512 * (ci + 1)],
                                        ktf[t][:, 128 * kk:128 * (kk + 1)],
                                        qd_sb[t][sl][:],
                                        start=True, stop=True,
                                    )
                                nc.scalar.activation(p_sb[:], s_ps[:],
                                                     mybir.ActivationFunctionType.Exp)
                                for ci, kk in enumerate(chunks):
                                    for par in (0, 1):
                                        # o_ps bank is shared by the head pair:
                                        # only the very first matmul clears
                                        # has_written
                                        nc.tensor.matmul(
                                            o_ps[0:VW, 256 * par:256 * (par + 1)],
                                            vtf[kk][:, VW * (2 * t + par):VW * (2 * t + par) + VW],
                                            p_sb[:, 512 * ci + 256 * par:512 * ci + 256 * par + 256],
                                            start=(g == 0 and ci == 0 and par == 0),
                                            stop=(g == s - 1 and ci == 1 and par == 1),
                                            skip_group_check=True,
                                        )
                            # combine with diagonal partial and normalize
                            if s == 0:
                                o_sum = o_diag[t][sl]
                            else:
                                o_sum = osum_ring[ni % 2]
                                nc.vector.tensor_add(o_sum[:], o_ps[0:VW, :],
                                                     o_diag[t][sl][:])
                            rc = rc_ring[ni % 2]
                            dn = dn_ring[ni % 2]
                            ni += 1
                            nc.vector.tensor_copy(dn[:], o_sum[64:65, :])
                            nc.tensor.matmul(o_ps[64:128, 0:256], ones_sb[:],
                                             dn[0:1, 0:256], start=True, stop=True)
                            nc.tensor.matmul(o_ps[64:128, 256:512], ones_sb[:],
                                             dn[0:1, 256:512], start=True, stop=True)
                            nc.vector.reciprocal(rc[:], o_ps[64:128, :])
                            for par in (0, 1):
                                nc.vector.tensor_mul(
                                    attnT_sb[t][64 * par:64 * par + 64,
                                                STR * sl:STR * (sl + 1)],
                                    o_sum[0:64, 256 * par:256 * (par + 1)],
                                    rc[:, 256 * par:256 * (par + 1)],
                                )

                        # output projection for this slot (overlaps next slot)
                        for qc in (2 * sl, 2 * sl + 1):
                            o_ps = o_ring[oi % 3]
                            oi += 1
                            o_sb = ob_ring[qc % 2]
                            for ec in range(6):
                                nc.tensor.matmul(
                                    o_ps[:, 0:512],
                                    attnT_sb[ec][:, 128 * qc:128 * (qc + 1)],
                                    woutT_sb[ec][:, 0:512],
                                    start=(ec == 0), stop=(ec == 5),
                                )
                            nc.vector.tensor_copy(o_sb[:, 0:512], o_ps[:, 0:512])
                            for ec in range(6):
                                nc.tensor.matmul(
                                    o_ps[:, 0:256],
                                    attnT_sb[ec][:, 128 * qc:128 * (qc + 1)],
                                    woutT_sb[ec][:, 512:768],
                                    start=(ec == 0), stop=(ec == 5),
                                )
                            nc.vector.tensor_copy(o_sb[:, 512:768], o_ps[:, 0:256])
                            nc.sync.dma_start(out_own[128 * qc:128 * (qc + 1), :], o_sb[:])

                # binary branch tree: each core evaluates 3 compares, not 8
                with tc.If(pid < 4) as c_lo:
                    with tc.If(pid < 2) as c_01:
                        with tc.If(pid < 1) as c_0:
                            attention_body(0)
                        with c_0.Else():
                            attention_body(1)
                    with c_01.Else():
                        with tc.If(pid < 3) as c_2:
                            attention_body(2)
                        with c_2.Else():
                            attention_body(3)
                with c_lo.Else():
                    with tc.If(pid < 6) as c_45:
                        with tc.If(pid < 5) as c_4:
                            attention_body(4)
                        with c_4.Else():
                            attention_body(5)
                    with c_45.Else():
                        with tc.If(pid < 7) as c_6:
                            attention_body(6)
                        with c_6.Else():
                            attention_body(7)

    nc.compile()
    return nc


_cache_lock = threading.Lock()
_cached_nc = None


def _get_program():
    global _cached_nc
    with _cache_lock:
        if _cached_nc is None:
            _cached_nc = build_program()
    return _cached_nc


def _host_fallback(x, mask, w_qkv, b_qkv, w_out, b_out):
    x2 = x.reshape(S, E).astype(np.float64)
    qkv = x2 @ w_qkv.T.astype(np.float64) + b_qkv
    q, k, v = np.split(qkv, 3, axis=-1)
    q = q.reshape(S, H, DH).transpose(1, 0, 2)
    k = k.reshape(S, H, DH).transpose(1, 0, 2)
    v = v.reshape(S, H, DH).transpose(1, 0, 2)
    out = np.empty((H, S, DH))
    for h in range(H):
        sc = q[h] @ k[h].T
        sc = np.where(mask, sc, -np.inf)
        sc = sc - sc.max(axis=-1, keepdims=True)
        p = np.exp(sc)
        p /= p.sum(axis=-1, keepdims=True)
        out[h] = p @ v[h]
    attn = out.transpose(1, 0, 2).reshape(S, E)
    res = attn @ w_out.T.astype(np.float64) + b_out
    return res.reshape(1, S, E).astype(np.float32)


def kernel(x, mask, w_qkv, b_qkv, w_out, b_out, _trace=False):
    x = np.asarray(x)
    mask = np.asarray(mask).astype(bool)
    w_qkv = np.asarray(w_qkv, dtype=np.float32)
    b_qkv = np.asarray(b_qkv, dtype=np.float32)
    w_out = np.asarray(w_out, dtype=np.float32)
    b_out = np.asarray(b_out, dtype=np.float32)

    if not np.array_equal(mask, np.tril(np.ones((S, S), dtype=bool))):
        return _host_fallback(x, mask, w_qkv, b_qkv, w_out, b_out)

    x2 = x.reshape(S, E).astype(np.float32)

    wq = np.zeros((SE, E3), dtype=np.float32)
    wq[:E] = w_qkv.T
    wq[E] = b_qkv
    wq_bf = wq.astype(NP_BF16)

    woutT_np = np.ascontiguousarray(w_out.T).astype(NP_BF16)

    xT_full = np.zeros((SE, S), dtype=np.float32)
    xT_full[:E] = x2.T
    xT_full[E] = 1.0
    xT_bf = xT_full.astype(NP_BF16)

    in_maps = []
    for i in range(N_CORES):
        s0, s1 = STRIPES[i]
        cols = np.r_[STR * s0:STR * (s0 + 1), STR * s1:STR * (s1 + 1)]
        # dbias: per stripe-slot, bias for the last (diagonal) 2-chunk group,
        # layout [k, 512*ci + 256*par + q] (duplicated across the head pair)
        db = np.zeros((2, 128, 1024), dtype=np.float32)
        for sl, s in enumerate((s0, s1)):
            qrows = slice(STR * s, STR * (s + 1))
            for ci in range(2):
                kk = 2 * s + ci
                msub = mask[qrows, 128 * kk:128 * (kk + 1)]       # [256 q, 128 k]
                bias = np.where(msub.T, 0.0, NEG)                 # [128 k, 256 q]
                db[sl, :, 512 * ci:512 * ci + 256] = bias
                db[sl, :, 512 * ci + 256:512 * ci + 512] = bias
        in_maps.append({
            "xT": np.ascontiguousarray(xT_bf[:, cols]),
            "wqkvT": wq_bf,
            "woutT": woutT_np,
            "dbias": db.astype(NP_BF16),
        })

    nc = _get_program()
    res = run_bass_kernel_spmd(nc, in_maps, core_ids=list(range(N_CORES)),
                               trace=_trace)

    out = np.empty((S, E), dtype=np.float32)
    for i in range(N_CORES):
        s0, s1 = STRIPES[i]
        r = res.results[i]["out_own"]
        out[STR * s0:STR * (s0 + 1)] = r[0:STR]
        out[STR * s1:STR * (s1 + 1)] = r[STR:2 * STR]
    out = out + b_out[None, :]
    result = out.reshape(1, S, E).astype(np.float32)
    if _trace:
        return result, res
    return result


# revision 14
# speedup vs baseline: 1.8072x; 1.0382x over previous
"""Trainium2 Bass kernel for nn_Attention_12403865551261.

Causal multi-head attention (B=1, S=4096, E=768, H=12, Dh=64, no 1/sqrt(dh)
scaling) sharded over 8 NeuronCores.

Sharding: queries are row-sharded in a causal-balanced zigzag — core i owns
the two 256-row stripes {i, 15-i} of the 16-stripe sequence, so every core
does the same total attention work. Each core projects Q/K/V for its own 512
rows (all projections produced directly in transposed layout so no on-device
transposes are needed), K^T and V are AllGathered across the 8 cores in
bf16, then each core runs causal softmax(Q K^T) V for its own rows (only
the lower-triangle key blocks) and the output projection.

Per-core causal structure (different kv extents per stripe) is expressed as
8 compute-only branch bodies on tc.If(partition_id == i); all DMAs and the
projection/collective/output phases are uniform across cores.

Layout trick: scores are computed transposed, S_T[k, q], with dh on the
contraction dim; softmax denominators come free from a ones-column appended
to V (row 64 of the attention-output accumulator). Attention output is
produced directly as attnT[e, q], which is exactly the lhsT the output
projection wants. Score matmuls contract over only 64 partitions, so the
two heads of a pair are issued at partition offsets 0/64 — the PE runs them
concurrently in different row-groups (and the full-width activity keeps the
HAM clock-gate warm).

Mask handling: the causal structure is static; only each stripe's last
(diagonal) key group gets an additive bias tile (0 / -30000, host data).
exp(s - 30000) == 0.0 exactly in f32. If the mask input is not causal, a
host fallback computes the reference directly.
"""
import sys
import threading

sys.path.insert(0, "/opt/trn_rl_repo")

import numpy as np
import ml_dtypes

import concourse.bacc as bacc
import concourse.tile as tile
import concourse.mybir as mybir
from concourse.bass_utils import run_bass_kernel_spmd

BF16 = mybir.dt.bfloat16
F32 = mybir.dt.float32
NP_BF16 = ml_dtypes.bfloat16

N_CORES = 8
S = 4096
E = 768
H = 12
DH = 64
E3 = 3 * E          # 2304
SE = 896            # contraction dim: 768 weights + 1 bias row + pad -> 7*128
KC = SE // 128      # 7 contraction chunks
ROWS = S // N_CORES  # 512 own rows per core
STR = 256           # stripe height
NEG = -30000.0
VW = 65             # V columns per head incl. ones-column

# core i owns stripes (i, 15-i); stripe s covers global rows [256s, 256s+256)
STRIPES = [(i, 15 - i) for i in range(N_CORES)]


def stripe_owner(j):
    """global stripe j -> (core, slot)"""
    c = min(j, 15 - j)
    return c, (0 if j < 8 else 1)


def build_program():
    nc = bacc.Bacc("TRN2", target_bir_lowering=False, debug=False,
                   num_devices=N_CORES)

    xT = nc.dram_tensor("xT", [SE, ROWS], BF16, kind="ExternalInput")
    wqkvT = nc.dram_tensor("wqkvT", [SE, E3], BF16, kind="ExternalInput")
    woutT = nc.dram_tensor("woutT", [E, E], BF16, kind="ExternalInput")
    dbias = nc.dram_tensor("dbias", [2, 128, 1024], BF16, kind="ExternalInput")
    out_own = nc.dram_tensor("out_own", [ROWS, E], F32, kind="ExternalOutput")

    with tile.TileContext(nc) as tc:
        with (
            tc.tile_pool(name="const", bufs=1) as cpool,
            tc.tile_pool(name="dram", bufs=1, space="DRAM") as dram,
        ):
            # block-diagonal Q per (pair, stripe-slot): [Qe|0 ; 0|Qo]
            qd_sb = [[cpool.tile([128, 512], BF16, name=f"qd{t}_{sl}")
                      for sl in range(2)] for t in range(6)]
            attnT_sb = [cpool.tile([128, ROWS], BF16, name=f"at{t}") for t in range(6)]
            woutT_sb = [cpool.tile([128, E], BF16, name=f"wo{t}") for t in range(6)]
            kt_own = [cpool.tile([128, ROWS], BF16, name=f"kt{t}") for t in range(6)]
            v_own = [cpool.tile([128, H * VW], BF16, name=f"v{t}") for t in range(4)]
            ones_sb = cpool.tile([1, 64], F32, name="ones")
            nc.vector.memset(ones_sb[:], 1.0)
            for t in range(6):
                nc.sync.dma_start(woutT_sb[t][:], woutT[128 * t:128 * (t + 1), :])
            db_sb = [cpool.tile([128, 1024], BF16, name=f"db{sl}") for sl in range(2)]
            for sl in range(2):
                nc.sync.dma_start(db_sb[sl][:], dbias[sl, :, :])

            # lo = slot-0 stripes (global stripes 0-7), hi = slot-1 (8-15)
            ktc = [dram.tile([E, STR], BF16, name=f"ktc{h}") for h in range(2)]
            vc = [dram.tile([STR, H * VW], BF16, name=f"vc{h}") for h in range(2)]
            kt_g = [dram.tile([N_CORES * E, STR], BF16, addr_space="Shared",
                              name=f"ktg{h}") for h in range(2)]
            v_g = [dram.tile([N_CORES * STR, H * VW], BF16, addr_space="Shared",
                             name=f"vg{h}") for h in range(2)]

            # ---------------- phase 1: QKV projection (own rows) -----------
            with (
                tc.tile_pool(name="proj", bufs=1) as ppool,
                tc.tile_pool(name="pq", bufs=2, space="PSUM") as pq,
                tc.tile_pool(name="pv", bufs=2, space="PSUM") as pv,
            ):
                w_sb = [ppool.tile([128, E3], BF16, name=f"w{t}") for t in range(KC)]
                x_sb = [ppool.tile([128, ROWS], BF16, name=f"x{t}") for t in range(KC)]
                for t in range(KC):
                    nc.sync.dma_start(w_sb[t][:], wqkvT[128 * t:128 * (t + 1), :])
                    nc.sync.dma_start(x_sb[t][:], xT[128 * t:128 * (t + 1), :])

                # K first so its all-gather starts while Q/V still project
                for oc in range(6, 12):
                    ps = pq.tile([128, ROWS], F32, tag="pq")
                    for kc in range(KC):
                        nc.tensor.matmul(
                            ps[:],
                            w_sb[kc][:, 128 * oc:128 * (oc + 1)],
                            x_sb[kc][:],
                            start=(kc == 0), stop=(kc == KC - 1),
                        )
                    t = oc - 6
                    nc.vector.tensor_copy(kt_own[t][:], ps[:])
                    for h in range(2):
                        nc.sync.dma_start(
                            ktc[h][128 * t:128 * (t + 1), :],
                            kt_own[t][:, STR * h:STR * (h + 1)])
                for h in range(2):
                    nc.gpsimd.collective_compute(
                        "AllGather", mybir.AluOpType.bypass,
                        replica_groups=[list(range(N_CORES))],
                        ins=[ktc[h][:]], outs=[kt_g[h][:]],
                    )

                for sc in range(4):
                    ps = pv.tile([128, E], F32, tag="pv")
                    for kc in range(KC):
                        nc.tensor.matmul(
                            ps[:, 0:512],
                            x_sb[kc][:, 128 * sc:128 * (sc + 1)],
                            w_sb[kc][:, 2 * E:2 * E + 512],
                            start=(kc == 0), stop=(kc == KC - 1),
                        )
                        nc.tensor.matmul(
                            ps[:, 512:768],
                            x_sb[kc][:, 128 * sc:128 * (sc + 1)],
                            w_sb[kc][:, 2 * E + 512:3 * E],
                            start=(kc == 0), stop=(kc == KC - 1),
                        )
                    nc.vector.memset(v_own[sc][:], 1.0)
                    for h in range(H):
                        nc.vector.tensor_copy(
                            v_own[sc][:, VW * h:VW * h + DH],
                            ps[:, DH * h:DH * (h + 1)],
                        )
                    nc.sync.dma_start(vc[sc // 2][128 * (sc % 2):128 * (sc % 2 + 1), :],
                                      v_own[sc][:])
                for h in range(2):
                    nc.gpsimd.collective_compute(
                        "AllGather", mybir.AluOpType.bypass,
                        replica_groups=[list(range(N_CORES))],
                        ins=[vc[h][:]], outs=[v_g[h][:]],
                    )

                for oc in range(6):
                    ps = pq.tile([128, ROWS], F32, tag="pq")
                    for kc in range(KC):
                        nc.tensor.matmul(
                            ps[:],
                            w_sb[kc][:, 128 * oc:128 * (oc + 1)],
                            x_sb[kc][:],
                            start=(kc == 0), stop=(kc == KC - 1),
                        )
                    for sl in range(2):
                        qd = qd_sb[oc][sl]
                        nc.vector.memset(qd[:], 0.0)
                        nc.vector.tensor_copy(
                            qd[0:64, 0:256], ps[0:64, STR * sl:STR * (sl + 1)])
                        nc.vector.tensor_copy(
                            qd[64:128, 256:512], ps[64:128, STR * sl:STR * (sl + 1)])

            # ---------------- phase 2: attention ---------------------------
            with (
                tc.tile_pool(name="attn", bufs=1) as apool,
                tc.tile_pool(name="ps_s", bufs=1, space="PSUM") as ps_s,
                tc.tile_pool(name="ps_o", bufs=1, space="PSUM") as ps_o,
            ):
                ktf = [apool.tile([128, S], BF16, name=f"ktf{t}") for t in range(6)]
                vtf = [apool.tile([128, H * VW], BF16, name=f"vtf{kk}") for kk in range(32)]
                s_ring = [ps_s.tile([128, 1536], F32, name=f"sr{j}", tag=f"sr{j}")
                          for j in range(2)]
                o_ring = [ps_o.tile([128, 512], F32, name=f"or{j}", tag=f"or{j}")
                          for j in range(2)]
                p_ring = [apool.tile([128, 1536], BF16, name=f"pr{j}") for j in range(2)]
                rc_ring = [apool.tile([64, 512], F32, name=f"rcr{j}") for j in range(2)]
                dn_ring = [apool.tile([1, 512], F32, name=f"dn{j}") for j in range(2)]
                o_diag = [[apool.tile([VW, 512], F32, name=f"od{t}_{sl}")
                           for sl in range(2)] for t in range(6)]
                o_cross = [apool.tile([VW, 512], F32, name=f"oc{t}") for t in range(6)]
                ob_ring = [apool.tile([128, E], F32, name=f"ob{j}") for j in range(2)]

                # --- own-data precompute: uniform across cores, overlaps the
                # AllGathers. (a) diagonal groups for both stripes; (b) the
                # "cross" block: slot-1 queries x own slot-0 keys ------------
                dsi = 0
                doi = 0

                def own_block(t, sl_q, kcol, bias, dest):
                    nonlocal dsi, doi
                    s_ps = s_ring[dsi % 2]
                    p_sb = p_ring[dsi % 2]
                    dsi += 1
                    for ci in range(2):
                        nc.tensor.matmul(
                            s_ps[:, 512 * ci:512 * (ci + 1)],
                            kt_own[t][:, kcol + 128 * ci:kcol + 128 * (ci + 1)],
                            qd_sb[t][sl_q][:],
                            start=True, stop=True,
                        )
                    if bias is not None:
                        nc.vector.tensor_add(s_ps[:, 0:1024], s_ps[:, 0:1024],
                                             bias[:])
                    nc.scalar.activation(p_sb[:, 0:1024], s_ps[:, 0:1024],
                                         mybir.ActivationFunctionType.Exp)
                    o_ps = o_ring[doi % 2]
                    doi += 1
                    for ci in range(2):
                        for par in (0, 1):
                            nc.tensor.matmul(
                                o_ps[0:VW, 256 * par:256 * (par + 1)],
                                v_own[(kcol // 128) + ci][:, VW * (2 * t + par):VW * (2 * t + par) + VW],
                                p_sb[:, 512 * ci + 256 * par:512 * ci + 256 * par + 256],
                                start=(ci == 0 and par == 0),
                                stop=(ci == 1 and par == 1),
                                skip_group_check=True,
                            )
                    nc.vector.tensor_copy(dest[:], o_ps[0:VW, :])

                for t in range(6):
                    own_block(t, 0, 0, db_sb[0], o_diag[t][0])      # slot-0 diag
                    own_block(t, 1, STR, db_sb[1], o_diag[t][1])    # slot-1 diag
                    own_block(t, 1, 0, None, o_cross[t])            # cross block

                pid = nc.partition_id()

                def attention_body(core):
                    si = 0
                    oi = 0
                    ni = 0
                    s0, s1 = STRIPES[core]
                    # gather loads inside the branch so they overlap compute
                    for j in range(16):              # global stripe j, low keys first
                        h, rank = (0, j) if j < 8 else (1, 15 - j)
                        for t in range(6):
                            nc.sync.dma_start(
                                ktf[t][:, STR * j:STR * (j + 1)],
                                kt_g[h][E * rank + 128 * t:E * rank + 128 * (t + 1), :],
                            )
                        for half in range(2):
                            nc.sync.dma_start(
                                vtf[2 * j + half][:],
                                v_g[h][STR * rank + 128 * half:STR * rank + 128 * (half + 1), :])

                    def pair_stripe(t, sl):
                        nonlocal si, oi, ni
                        s = STRIPES[core][sl]
                        # main chunks: all strictly-below-diagonal key chunks,
                        # minus the own-cross block (slot 1 excludes stripe s0)
                        if sl == 1:
                            chunks = [c for c in range(2 * s)
                                      if c not in (2 * s0, 2 * s0 + 1)]
                        else:
                            chunks = list(range(2 * s))
                        groups = [chunks[j:j + 3] for j in range(0, len(chunks), 3)]
                        o_ps = o_ring[oi % 2]
                        oi += 1
                        for gi, grp in enumerate(groups):
                            W = 512 * len(grp)
                            s_ps = s_ring[si % 2]
                            p_sb = p_ring[si % 2]
                            si += 1
                            for ci, kk in enumerate(grp):
                                nc.tensor.matmul(
                                    s_ps[:, 512 * ci:512 * (ci + 1)],
                                    ktf[t][:, 128 * kk:128 * (kk + 1)],
                                    qd_sb[t][sl][:],
                                    start=True, stop=True,
                                )
                            nc.scalar.activation(p_sb[:, 0:W], s_ps[:, 0:W],
                                                 mybir.ActivationFunctionType.Exp)
                            for ci, kk in enumerate(grp):
                                for par in (0, 1):
                                    # o_ps bank shared by the head pair: only
                                    # the very first matmul clears has_written
                                    nc.tensor.matmul(
                                        o_ps[0:VW, 256 * par:256 * (par + 1)],
                                        vtf[kk][:, VW * (2 * t + par):VW * (2 * t + par) + VW],
                                        p_sb[:, 512 * ci + 256 * par:512 * ci + 256 * par + 256],
                                        start=(gi == 0 and ci == 0 and par == 0),
                                        stop=(gi == len(groups) - 1 and ci == len(grp) - 1 and par == 1),
                                        skip_group_check=True,
                                    )
                        # fold partials into o_diag (in place), then normalize
                        o_sum = o_diag[t][sl]
                        if chunks:
                            nc.vector.tensor_add(o_sum[:], o_ps[0:VW, :], o_sum[:])
                        if sl == 1:
                            nc.vector.tensor_add(o_sum[:], o_cross[t][:], o_sum[:])
                        rc = rc_ring[ni % 2]
                        dn = dn_ring[ni % 2]
                        ni += 1
                        nc.vector.tensor_copy(dn[:], o_sum[64:65, :])
                        nc.tensor.matmul(o_ps[64:128, 0:256], ones_sb[:],
                                         dn[0:1, 0:256], start=True, stop=True)
                        nc.tensor.matmul(o_ps[64:128, 256:512], ones_sb[:],
                                         dn[0:1, 256:512], start=True, stop=True)
                        nc.vector.reciprocal(rc[:], o_ps[64:128, :])
                        for par in (0, 1):
                            nc.vector.tensor_mul(
                                attnT_sb[t][64 * par:64 * par + 64,
                                            STR * sl:STR * (sl + 1)],
                                o_sum[0:64, 256 * par:256 * (par + 1)],
                                rc[:, 256 * par:256 * (par + 1)],
                            )

                    for t in range(6):
                        pair_stripe(t, 1)   # big stripe: dense PE work
                        pair_stripe(t, 0)   # small stripe rides along

                    # output projection
                    for qc in range(4):
                        o_ps = o_ring[oi % 2]
                        oi += 1
                        o_sb = ob_ring[qc % 2]
                        for ec in range(6):
                            nc.tensor.matmul(
                                o_ps[:, 0:512],
                                attnT_sb[ec][:, 128 * qc:128 * (qc + 1)],
                                woutT_sb[ec][:, 0:512],
                                start=(ec == 0), stop=(ec == 5),
                            )
                        nc.vector.tensor_copy(o_sb[:, 0:512], o_ps[:, 0:512])
                        for ec in range(6):
                            nc.tensor.matmul(
                                o_ps[:, 0:256],
                                attnT_sb[ec][:, 128 * qc:128 * (qc + 1)],
                                woutT_sb[ec][:, 512:768],
                                start=(ec == 0), stop=(ec == 5),
                            )
                        nc.vector.tensor_copy(o_sb[:, 512:768], o_ps[:, 0:256])
                        nc.sync.dma_start(out_own[128 * qc:128 * (qc + 1), :], o_sb[:])

                # binary branch tree: each core evaluates 3 compares, not 8
                with tc.If(pid < 4) as c_lo:
                    with tc.If(pid < 2) as c_01:
                        with tc.If(pid < 1) as c_0:
                            attention_body(0)
                        with c_0.Else():
                            attention_body(1)
                    with c_01.Else():
                        with tc.If(pid < 3) as c_2:
                            attention_body(2)
                        with c_2.Else():
                            attention_body(3)
                with c_lo.Else():
                    with tc.If(pid < 6) as c_45:
                        with tc.If(pid < 5) as c_4:
                            attention_body(4)
                        with c_4.Else():
                            attention_body(5)
                    with c_45.Else():
                        with tc.If(pid < 7) as c_6:
                            attention_body(6)
                        with c_6.Else():
                            attention_body(7)

    nc.compile()
    return nc


_cache_lock = threading.Lock()
_cached_nc = None


def _get_program():
    global _cached_nc
    with _cache_lock:
        if _cached_nc is None:
            _cached_nc = build_program()
    return _cached_nc


def _host_fallback(x, mask, w_qkv, b_qkv, w_out, b_out):
    x2 = x.reshape(S, E).astype(np.float64)
    qkv = x2 @ w_qkv.T.astype(np.float64) + b_qkv
    q, k, v = np.split(qkv, 3, axis=-1)
    q = q.reshape(S, H, DH).transpose(1, 0, 2)
    k = k.reshape(S, H, DH).transpose(1, 0, 2)
    v = v.reshape(S, H, DH).transpose(1, 0, 2)
    out = np.empty((H, S, DH))
    for h in range(H):
        sc = q[h] @ k[h].T
        sc = np.where(mask, sc, -np.inf)
        sc = sc - sc.max(axis=-1, keepdims=True)
        p = np.exp(sc)
        p /= p.sum(axis=-1, keepdims=True)
        out[h] = p @ v[h]
    attn = out.transpose(1, 0, 2).reshape(S, E)
    res = attn @ w_out.T.astype(np.float64) + b_out
    return res.reshape(1, S, E).astype(np.float32)


def kernel(x, mask, w_qkv, b_qkv, w_out, b_out, _trace=False):
    x = np.asarray(x)
    mask = np.asarray(mask).astype(bool)
    w_qkv = np.asarray(w_qkv, dtype=np.float32)
    b_qkv = np.asarray(b_qkv, dtype=np.float32)
    w_out = np.asarray(w_out, dtype=np.float32)
    b_out = np.asarray(b_out, dtype=np.float32)

    if not np.array_equal(mask, np.tril(np.ones((S, S), dtype=bool))):
        return _host_fallback(x, mask, w_qkv, b_qkv, w_out, b_out)

    x2 = x.reshape(S, E).astype(np.float32)

    wq = np.zeros((SE, E3), dtype=np.float32)
    wq[:E] = w_qkv.T
    wq[E] = b_qkv
    wq_bf = wq.astype(NP_BF16)

    woutT_np = np.ascontiguousarray(w_out.T).astype(NP_BF16)

    xT_full = np.zeros((SE, S), dtype=np.float32)
    xT_full[:E] = x2.T
    xT_full[E] = 1.0
    xT_bf = xT_full.astype(NP_BF16)

    in_maps = []
    for i in range(N_CORES):
        s0, s1 = STRIPES[i]
        cols = np.r_[STR * s0:STR * (s0 + 1), STR * s1:STR * (s1 + 1)]
        # dbias: per stripe-slot, bias for the last (diagonal) 2-chunk group,
        # layout [k, 512*ci + 256*par + q] (duplicated across the head pair)
        db = np.zeros((2, 128, 1024), dtype=np.float32)
        for sl, s in enumerate((s0, s1)):
            qrows = slice(STR * s, STR * (s + 1))
            for ci in range(2):
                kk = 2 * s + ci
                msub = mask[qrows, 128 * kk:128 * (kk + 1)]       # [256 q, 128 k]
                bias = np.where(msub.T, 0.0, NEG)                 # [128 k, 256 q]
                db[sl, :, 512 * ci:512 * ci + 256] = bias
                db[sl, :, 512 * ci + 256:512 * ci + 512] = bias
        in_maps.append({
            "xT": np.ascontiguousarray(xT_bf[:, cols]),
            "wqkvT": wq_bf,
            "woutT": woutT_np,
            "dbias": db.astype(NP_BF16),
        })

    nc = _get_program()
    res = run_bass_kernel_spmd(nc, in_maps, core_ids=list(range(N_CORES)),
                               trace=_trace)

    out = np.empty((S, E), dtype=np.float32)
    for i in range(N_CORES):
        s0, s1 = STRIPES[i]
        r = res.results[i]["out_own"]
        out[STR * s0:STR * (s0 + 1)] = r[0:STR]
        out[STR * s1:STR * (s1 + 1)] = r[STR:2 * STR]
    out = out + b_out[None, :]
    result = out.reshape(1, S, E).astype(np.float32)
    if _trace:
        return result, res
    return result
